# revision 52
# baseline (speedup 1.0000x reference)
"""GQA attention kernel for 8 TRN2 NeuronCores (tensor-parallel over heads).

Problem: B=2, S=2048, D=2048, HQ=32, HKV=8, HD=64, ALiBi + additive mask,
softmax, out-projection.  Each core owns 4 q-heads (= 1 kv head); each core
computes a full-shape partial of the output (its heads' contribution through
wo), and the host sums the 8 partials.

v2 layout strategy (per core):
  - projections run as fp8e4 DoubleRow matmuls on host-prepared (hi, lo)
    residual pairs of x and wqkv (hi*hi + cross terms), which the cost
    model rates at 0.5 cycles/row; weights are pre-scaled by 1024 so the
    lo residuals stay in e4m3's normal range, undone during eviction.
  - everything else in bf16 (wo, v, exp(logits), attention outputs, DRAM
    output partial); psum stays f32.  ALiBi aug rows need f32 range
    (slope*m up to ~2e3), so the logits matmul runs f32r on f32 qaug/kaug
    whose data rows are written from the f32 projection psum.
  - logits computed TRANSPOSED: logitsT[n, m] = kaug.T @ qaug with the
    contraction dim augmented by 2 rows that add alibi slope*(n-m) and a
    per-query stabilizer -c[m] for free:
       kaug = [kT(64); n; 1]            (shared by all 4 heads)
       qaug_h = [qT_h(64); slope_h; -slope_h*m - c_h[m]]
  - attention is pipelined per 512-query chunk: logits -> exp(ACT, bf16
    out) with the AV matmuls emitted LAG chunk-steps later so the exp
    round-trip latency never blocks the in-order PE queue.  qk psum tiles
    rotate through a 4-deep ring; all other psum users (kv-projection,
    v-transpose, AV accumulators, out-projection) share a second 4-deep
    ring of banks, placed so no allocation ever waits on a slow eviction.
  - AV matmul uses vaug = [v | ones] so the ones column accumulates
    softmax denominators in psum row 64.  AV matmuls are column-trimmed
    to the causal region with per-diagonal-block stop flags.
  - normalization: denominator row is copied out of psum (DVE),
    reciprocal'd in place, partition-broadcast (Pool), and multiplied
    into the bf16 psum eviction (DVE).  Odd heads are DMA-shifted to
    partitions 64:127 so the o-projection reads one contiguous [128, m]
    stationary per head pair.
  - out-projection is split into per-128-query units and software-
    pipelined: units are interleaved into the NEXT attention/projection
    phase so the PE never waits on the normalize chain.
  - causal masks: dead logit tiles are skipped; diagonal-crossing tiles
    accumulate a precomputed [128,128] additive pattern on the PE itself
    (ident128.T @ mpat in bf16, 53ns) instead of a DVE pass.
"""

import sys

sys.path.insert(0, "/opt/trn_rl_repo")

import numpy as np

NEG = -1e9


# ---------------------------------------------------------------------------
# device program builder
# ---------------------------------------------------------------------------

def build_program(cfg):
    import concourse.bass as bass  # noqa: F401
    import concourse.mybir as mybir
    import concourse.tile as tile
    from concourse import bacc

    f32 = mybir.dt.float32
    f32r = mybir.dt.float32r
    bf16 = mybir.dt.bfloat16

    B, S, D = cfg["B"], cfg["S"], cfg["D"]
    HLOC, HD = cfg["HLOC"], cfg["HD"]
    MC = cfg["MC"]                    # m-chunk (<= 512, psum bank)
    MPAIR = 2 * MC                    # exp / AV / normalize granularity
    causal = cfg["causal"]
    generic_mask = cfg["generic_mask"]

    DQ = HLOC * HD                    # local q dims (256)
    DKV = 2 * HD                      # local kv dims (128)
    NKT = D // 128                    # contraction k-tiles for projections
    NNT = S // 128                    # n-tiles (keys)
    NMC = S // MC                     # m-chunks per b
    NPAIR = S // MPAIR                # m-pairs per b
    NHP = HLOC // 2                   # head pairs
    NEC = D // MC                     # out-proj e-chunks

    nc = bacc.Bacc("TRN2", target_bir_lowering=False, debug=False)

    fp8 = mybir.dt.float8e4
    WPAD = 512                        # wqkv cols padded for 512B dma elems
    # x and wqkv as fp8 (hi, lo) residual pairs for DoubleRow matmuls;
    # layouts [kt, slot, p, cols]; w slots (0=lo, 1=hi), x slots (0=hi, 1=lo)
    xT_d = nc.dram_tensor("xT2", [NKT, 2, 128, B, S], fp8,
                          kind="ExternalInput")
    wqkv_d = nc.dram_tensor("wqkv2", [NKT, 2, 128, WPAD], fp8,
                            kind="ExternalInput")
    wo_d = nc.dram_tensor("woT", [DQ, D], bf16, kind="ExternalInput")
    kaug_d = nc.dram_tensor("kaug_ext", [2, S], f32, kind="ExternalInput")
    qaug_d = nc.dram_tensor("qaug_ext", [HLOC, 2, S], f32, kind="ExternalInput")
    ident_d = nc.dram_tensor("ident", [64, 64], bf16, kind="ExternalInput")
    if causal:
        # mask pattern applied on the PE: qk += ident128.T @ mpat
        ident128_d = nc.dram_tensor("ident128", [128, 128], bf16,
                                    kind="ExternalInput")
        mpat_d = nc.dram_tensor("maskpat", [128, 128], bf16,
                                kind="ExternalInput")
    if generic_mask:
        maskT_d = nc.dram_tensor("maskT", [S, S], f32, kind="ExternalInput")
    out_d = nc.dram_tensor("out", [B, S, D], bf16, kind="ExternalOutput")

    def live(nt, mc):
        """is logitsT tile (keys nt*128.., queries mc*MC..) not fully masked"""
        if not causal:
            return True
        return nt * 128 <= mc * MC + MC - 1

    def crossing(nt, mc):
        """does the tile cross the causal diagonal (needs mask pattern)"""
        if not causal:
            return False
        return live(nt, mc) and nt * 128 + 127 > mc * MC

    with tile.TileContext(nc) as tc:
        with tc.tile_pool(name="res", bufs=1) as res, \
             tc.tile_pool(name="dbl", bufs=2) as dbl, \
             tc.tile_pool(name="sbp", bufs=3) as sbp, \
             tc.tile_pool(name="ps", bufs=1, space="PSUM") as ps:

            # ---- resident weights ----------------------------------------
            # wqkv quarters go on the SP queue (needed by the first matmul);
            # everything else loads via the ACT queue so the first xt DMA
            # isn't stuck behind resident loads on the in-order SP queue.
            wqkv_sb = res.tile([128, NKT, 2, WPAD], fp8, tag="wqkv")
            qtr = NKT // 4

            def _wqkv_quarter(qi):
                nc.sync.dma_start(
                    wqkv_sb[:, qi * qtr:(qi + 1) * qtr, :, :],
                    wqkv_d.ap()[qi * qtr:(qi + 1) * qtr]
                    .rearrange("kt two p q -> p kt two q"))

            # quarter 0 now; 1-3 deferred until after the first xt DMA so the
            # first projection matmul isn't stuck behind them on DMA_ENGINES
            _wqkv_quarter(0)
            deferred = [lambda qi=qi: _wqkv_quarter(qi) for qi in range(1, 4)]
            wo_sb = res.tile([128, NHP, D], bf16, tag="wo")
            ident_sb = res.tile([64, 64], bf16, tag="ident")
            if causal:
                ident128_sb = res.tile([128, 128], bf16, tag="ident128")
                mpat_sb = res.tile([128, 128], bf16, tag="mpat")

            def _load_misc():
                nc.scalar.dma_start(
                    wo_sb[:],
                    wo_d.ap()[:].rearrange("(hp p) e -> p hp e", p=128))
                nc.scalar.dma_start(ident_sb[:], ident_d.ap()[:])
                if causal:
                    nc.scalar.dma_start(ident128_sb[:], ident128_d.ap()[:])
                    nc.scalar.dma_start(mpat_sb[:], mpat_d.ap()[:])

            deferred.append(lambda: _load_misc())

            # per-b double-buffered activations (allocated inside the b loop)
            state = {}
            alt = {"i": 0}  # DVE/Pool alternation for mask adds + oproj evicts

            def proj_mc(b, mc):
                """projections for m-chunk mc of batch b"""
                kaug, qaug, vt = state["kaug"], state["qaug"], state["vt"]
                mco = mc * MC
                qp = [ps.tile([128, MC], f32, tag="qk", bufs=4,
                              name=f"qp{hp}") for hp in range(NHP)]
                kvp = ps.tile([128, MC], f32, tag="ps4", bufs=4, name="kvp")
                KQ = 4  # k-tiles per xt DMA
                DR = mybir.MatmulPerfMode.DoubleRow
                for ktq in range(NKT // KQ):
                    xt = sbp.tile([128, KQ, 2, MC], fp8, tag="xt", bufs=6)
                    nc.sync.dma_start(
                        xt[:], xT_d.ap()[ktq * KQ:(ktq + 1) * KQ, :, :,
                                         b, mco:mco + MC]
                        .rearrange("kt two p m -> p kt two m"))
                    while deferred:
                        deferred.pop(0)()
                    st = (ktq == 0)
                    sp = (ktq == NKT // KQ - 1)
                    groups = [(qp[0], 0), (qp[1], 128), (kvp, DQ)]
                    for dst, g0 in groups:
                        csl = slice(g0, g0 + 128) if g0 < DQ                             else slice(DQ, DQ + DKV)
                        # hi*hi over kt pairs
                        for kp in range(KQ // 2):
                            nc.tensor.matmul(
                                dst[:],
                                wqkv_sb[:, ktq * KQ + 2 * kp:
                                        ktq * KQ + 2 * kp + 2, 1, csl],
                                xt[:, 2 * kp:2 * kp + 2, 0, :],
                                start=st and kp == 0, stop=False,
                                perf_mode=DR)
                        # cross terms (w_lo x_hi + w_hi x_lo) per kt
                        for kq in range(KQ):
                            nc.tensor.matmul(
                                dst[:],
                                wqkv_sb[:, ktq * KQ + kq, :, csl],
                                xt[:, kq, :, :],
                                start=False, stop=sp and kq == KQ - 1,
                                perf_mode=DR)
                # evictions, spread across DVE/ACT so qp frees fast
                # (GPSIMD cannot access PSUM)
                WS = 1.0 / 1024.0  # undo the fp8 weight scaling
                for hp in range(NHP):
                    # even head of the pair: psum rows 0:64 -> qaug rows 0:64
                    nc.vector.tensor_scalar_mul(
                        qaug[2 * hp][0:64, mco:mco + MC], qp[hp][0:64, :], WS)
                    # odd head: rows 64:128, engine-copy then DMA shift
                    qtmp = sbp.tile([128, MC], f32r, tag="qtmp", bufs=4,
                                    name="qtmp")
                    nc.vector.tensor_scalar_mul(qtmp[64:128, :],
                                                qp[hp][64:128, :], WS)
                    nc.sync.dma_start(qaug[2 * hp + 1][0:64, mco:mco + MC],
                                      qtmp[64:128, :])
                nc.vector.tensor_scalar_mul(kaug[0:64, mco:mco + MC],
                                            kvp[0:64, :], WS)
                vtmp = sbp.tile([128, MC], bf16, tag="vtmp", bufs=2,
                                name="vtmp")
                nc.scalar.activation(vtmp[64:128, :], kvp[64:128, :],
                                     mybir.ActivationFunctionType.Copy,
                                     scale=WS)
                nc.sync.dma_start(vt[0:64, mco:mco + MC], vtmp[64:128, :])

            def vtrans(b):
                """transpose vT -> v (vaug), groups of 8 n-tiles per psum"""
                vt, vaug = state["vt"], state["vaug"]
                for g in range((NNT + 7) // 8):
                    nts = range(g * 8, min((g + 1) * 8, NNT))
                    vtp = ps.tile([128, 512], bf16, tag="ps4", bufs=4,
                                  name="vtp")
                    for j, nt in enumerate(nts):
                        nc.tensor.transpose(
                            vtp[:, j * 64:(j + 1) * 64],
                            vt[0:64, nt * 128:(nt + 1) * 128], ident_sb[:])
                    nc.vector.tensor_copy(vaug[:, nts.start:nts.stop, 0:HD],
                                          vtp[:, 0:64 * len(nts)].rearrange(
                                              "p (t d) -> p t d", d=64))

            def attn_head(b, pair, h):
                kaug, qaug, vaug = state["kaug"], state["qaug"], state["vaug"]
                OT = state["OT"]
                hp, odd = h // 2, h % 2
                av = [ps.tile([128, MC], f32, tag="ps4", bufs=4,
                              name=f"av{c}") for c in range(2)]
                nlive = [nt for nt in range(NNT)
                         if live(nt, 2 * pair) or live(nt, 2 * pair + 1)]
                last_nt = nlive[-1]

                def emit_av(nt, c, pt):
                    st = (nt == 0)
                    mc = 2 * pair + c
                    if causal:
                        # columns whose diagonal (last) tile is nt
                        sl = max(0, nt * 128 - mc * MC)
                        sh = min(MC, nt * 128 + 128 - mc * MC)
                        if sh > sl:
                            nc.tensor.matmul(
                                av[c][0:HD + 1, sl:sh],
                                vaug[:, nt, :], pt[:, sl:sh],
                                start=st, stop=True,
                                skip_group_check=True)
                            if sh < MC:
                                nc.tensor.matmul(
                                    av[c][0:HD + 1, sh:MC],
                                    vaug[:, nt, :], pt[:, sh:MC],
                                    start=st, stop=False,
                                    skip_group_check=True)
                        else:
                            nc.tensor.matmul(
                                av[c][0:HD + 1, :], vaug[:, nt, :], pt[:],
                                start=st, stop=False,
                                skip_group_check=True)
                    else:
                        nc.tensor.matmul(
                            av[c][0:HD + 1, :], vaug[:, nt, :], pt[:],
                            start=st, stop=(nt == last_nt))

                # software pipeline: AV runs LAG chunk-steps behind
                # logits/exp so the exp round-trip latency never blocks the
                # in-order PE queue
                LAG = 6
                fifo = []
                for nt in nlive:
                    for c in range(2):
                        mc = 2 * pair + c
                        if not live(nt, mc):
                            continue
                        o = max(0, nt * 128 - mc * MC) if causal else 0
                        qk = ps.tile([128, MC], f32, tag="qk", bufs=4,
                                     name="qk")
                        pt = sbp.tile([128, MC], bf16, tag="pt", bufs=8,
                                      name="pt")
                        cross = crossing(nt, mc)
                        nc.tensor.matmul(
                            qk[:, o:MC],
                            kaug[:, nt * 128:(nt + 1) * 128],
                            qaug[h][:, mc * MC + o:(mc + 1) * MC],
                            start=True, stop=not cross,
                            skip_group_check=cross)
                        if generic_mask:
                            mtile = sbp.tile([128, MC], f32, tag="mt",
                                             name="mt")
                            nc.sync.dma_start(
                                mtile[:],
                                maskT_d.ap()[nt * 128:(nt + 1) * 128,
                                             mc * MC:(mc + 1) * MC])
                            nc.vector.tensor_add(qk[:], qk[:], mtile[:])
                        elif cross:
                            # accumulate the causal pattern on the PE
                            nc.tensor.matmul(
                                qk[:, o:o + 128], ident128_sb[:],
                                mpat_sb[:], start=False, stop=True,
                                skip_group_check=True)
                        nc.scalar.activation(
                            pt[:, o:MC], qk[:, o:MC],
                            mybir.ActivationFunctionType.Exp)
                        fifo.append((nt, c, pt))
                        if len(fifo) > LAG:
                            emit_av(*fifo.pop(0))
                for item in fifo:
                    emit_av(*item)
                # normalize per chunk (chunk 0's chain overlaps chunk 1's
                # remaining AV matmuls): denom row -> broadcast -> divide
                # folded into the bf16 psum eviction
                qdiv = None
                if odd:
                    qdiv = sbp.tile([64, MPAIR], bf16, tag="qdiv", bufs=2,
                                    name="qdiv")
                for c in range(2):
                    cs = slice(c * MC, (c + 1) * MC)
                    srow = sbp.tile([1, MC], f32, tag="srow", bufs=6,
                                    name="srow")
                    nc.vector.tensor_copy(srow[0:1, :], av[c][HD:HD + 1, :])
                    nc.vector.reciprocal(srow[0:1, :], srow[0:1, :])
                    rbc = sbp.tile([128, MC], f32, tag="rbc", bufs=6,
                                   name="rbc")
                    nc.gpsimd.partition_broadcast(rbc[:], srow[0:1, :])
                    dst = qdiv[0:64, cs] if odd else OT[0:64, hp, cs]
                    nc.vector.tensor_mul(dst, av[c][0:64, :], rbc[0:64, :])
                if odd:
                    nc.sync.dma_start(OT[64:128, hp, :], qdiv[0:64, :])

            def oproj_mt(b, pair, OT, mtl, on_act):
                """out-projection for one 128-query tile of a pair; evicts on
                ACT during proj-phase drains (DVE is the proj bottleneck)"""
                mt = pair * (MPAIR // 128) + mtl
                ob = sbp.tile([128, D], bf16, tag="ob", bufs=2, name="ob")
                for ec in range(NEC):
                    op = ps.tile([128, MC], f32, tag="ps4", bufs=4, name="op")
                    for hp in range(NHP):
                        nc.tensor.matmul(
                            op[:],
                            OT[:, hp, mtl * 128:(mtl + 1) * 128],
                            wo_sb[:, hp, ec * MC:(ec + 1) * MC],
                            start=(hp == 0), stop=(hp == NHP - 1))
                    if on_act:
                        nc.scalar.activation(
                            ob[:, ec * MC:(ec + 1) * MC], op[:],
                            mybir.ActivationFunctionType.Copy)
                    else:
                        nc.vector.tensor_copy(ob[:, ec * MC:(ec + 1) * MC],
                                              op[:])
                nc.sync.dma_start(
                    out_d.ap()[b, mt * 128:(mt + 1) * 128, :], ob[:])

            # ---- schedule: proj/attention with interleaved out-proj ------
            pending = []

            def drain(k, on_act=False):
                for _ in range(min(k, len(pending))):
                    pending.pop(0)(on_act)

            for _rep in range(cfg.get("reps", 1)):
                for b in range(B):
                    state["kaug"] = dbl.tile([66, S], f32r, tag="kaug",
                                             bufs=2, name="kaug")
                    nc.scalar.dma_start(state["kaug"][64:66, :],
                                        kaug_d.ap()[:].bitcast(f32r))
                    qaug = []
                    for h in range(HLOC):
                        t = dbl.tile([66, S], f32r, tag=f"qaug{h}", bufs=2,
                                     name=f"qaug{h}")
                        nc.scalar.dma_start(t[64:66, :],
                                            qaug_d.ap()[h].bitcast(f32r))
                        qaug.append(t)
                    state["qaug"] = qaug
                    state["vt"] = dbl.tile([64, S], bf16, tag="vt", bufs=2,
                                           name="vt")
                    state["vaug"] = dbl.tile([128, NNT, HD + 1], bf16,
                                             tag="vaug", bufs=2, name="vaug")
                    nc.vector.memset(state["vaug"][:], 1.0)

                    for mc in range(NMC):
                        proj_mc(b, mc)
                        drain(2)
                    vtrans(b)
                    drain(2)
                    for pair in range(NPAIR):
                        OT = dbl.tile([128, NHP, MPAIR], bf16, tag="OT",
                                      bufs=3, name="OT")
                        state["OT"] = OT
                        for i, h in enumerate(range(HLOC)):
                            attn_head(b, pair, h)
                            drain((0, 2, 3, 3)[i])
                        for mtl in range(MPAIR // 128):
                            pending.append(
                                lambda on_act, b=b, pair=pair, OT=OT,
                                mtl=mtl: oproj_mt(b, pair, OT, mtl, on_act))
                drain(len(pending))

    nc.compile()
    return nc


# ---------------------------------------------------------------------------
# host side
# ---------------------------------------------------------------------------

def _analyze_mask(mask2d, S):
    """classify mask; return (causal, zeros, n_lo, n_hi)"""
    masked = mask2d < -1e8
    if not masked.any():
        return False, True, np.zeros(S, np.int64), np.full(S, S - 1, np.int64)
    tri = np.triu(np.ones((S, S), bool), 1)
    if (masked == tri).all() and (mask2d[~masked] == 0).all():
        return True, False, np.zeros(S, np.int64), np.arange(S)
    allowed = ~masked
    # guard fully-masked rows (keep index 0; softmax row is garbage anyway)
    any_allowed = allowed.any(axis=1)
    idx = np.arange(S)[None, :]
    n_hi = np.where(any_allowed, np.where(allowed, idx, -1).max(axis=1), 0)
    n_lo = np.where(any_allowed, np.where(allowed, idx, S).min(axis=1), 0)
    return False, False, n_lo, n_hi


def _bf16(a):
    import ml_dtypes
    return np.ascontiguousarray(a).astype(ml_dtypes.bfloat16)


def _make_inputs_for_core(core, x, wq, wk, wv, wo, slopes, mask, cfg):
    B, S, D, HLOC, HD = cfg["B"], cfg["S"], cfg["D"], cfg["HLOC"], cfg["HD"]
    h0 = core * HLOC
    kv = core  # one kv head per core
    scale = 1.0 / np.sqrt(HD)

    import ml_dtypes
    FP8 = ml_dtypes.float8_e4m3
    NKT = D // 128
    DQ, DKV = HLOC * HD, 2 * HD
    WPAD = 512

    def _fp8_pair(a):
        hi = a.astype(FP8)
        lo = (a - hi.astype(np.float32)).astype(FP8)
        return hi, lo

    xT = np.ascontiguousarray(x.transpose(2, 0, 1))                 # [D,B,S]
    x_hi, x_lo = _fp8_pair(xT.reshape(NKT, 128, B, S))
    xT2 = np.stack([x_hi, x_lo], axis=1)                   # [kt,2,p,B,S]
    wqkvT = np.concatenate(
        [wq[h0 * HD:(h0 + HLOC) * HD] * scale,
         wk[kv * HD:(kv + 1) * HD],
         wv[kv * HD:(kv + 1) * HD]], axis=0).T                      # [D,384]
    wpad = np.zeros((D, WPAD), np.float32)
    wpad[:, :DQ + DKV] = wqkvT * 1024.0
    w_hi, w_lo = _fp8_pair(wpad.reshape(NKT, 128, WPAD))
    wqkv2 = np.stack([w_lo, w_hi], axis=1)                 # [kt,2,p,512]
    woT = np.ascontiguousarray(wo[:, h0 * HD:(h0 + HLOC) * HD].T)   # [DQ,D]

    n = np.arange(S, dtype=np.float32)
    kaug_ext = np.stack([n, np.ones(S, np.float32)])                # [2,S]

    qaug_ext = np.zeros((HLOC, 2, S), np.float32)
    for i in range(HLOC):
        sl = float(slopes[h0 + i])
        # stabilizer c[m] = max over allowed n of slope*(n-m), clipped >= 0
        c = np.maximum(0.0, np.maximum(sl * (cfg["n_hi"] - n),
                                       sl * (cfg["n_lo"] - n)))
        qaug_ext[i, 0, :] = sl
        qaug_ext[i, 1, :] = -sl * n - c

    ident = np.eye(64, dtype=np.float32)

    ins = {"xT2": xT2, "wqkv2": wqkv2, "woT": _bf16(woT),
           "kaug_ext": kaug_ext, "qaug_ext": qaug_ext,
           "ident": _bf16(ident)}
    if cfg["causal"]:
        ii = np.arange(128)[:, None]
        jj = np.arange(128)[None, :]
        ins["maskpat"] = _bf16(np.where(ii > jj, NEG, 0.0))
        ins["ident128"] = _bf16(np.eye(128))
    if cfg["generic_mask"]:
        ins["maskT"] = np.ascontiguousarray(mask[0, 0].T)
    return ins


def kernel(x, wq, wk, wv, wo, slopes, mask):
    from concourse.bass_utils import run_bass_kernel_spmd

    x = np.asarray(x, dtype=np.float32)
    wq = np.asarray(wq, dtype=np.float32)
    wk = np.asarray(wk, dtype=np.float32)
    wv = np.asarray(wv, dtype=np.float32)
    wo = np.asarray(wo, dtype=np.float32)
    slopes = np.asarray(slopes, dtype=np.float32)
    mask = np.asarray(mask, dtype=np.float32)

    B, S, D = x.shape
    HQ = 32
    HD = D // HQ
    n_cores = 8
    HLOC = HQ // n_cores

    causal, zeros, n_lo, n_hi = _analyze_mask(mask[0, 0], S)
    cfg = dict(B=B, S=S, D=D, HLOC=HLOC, HD=HD, MC=512,
               causal=causal, generic_mask=not (causal or zeros),
               n_lo=n_lo, n_hi=n_hi)

    nc = build_program(cfg)
    in_maps = [_make_inputs_for_core(c, x, wq, wk, wv, wo, slopes, mask, cfg)
               for c in range(n_cores)]
    res = run_bass_kernel_spmd(nc, in_maps, core_ids=list(range(n_cores)))
    out = np.zeros((B, S, D), np.float32)
    for c in range(n_cores):
        out += res.results[c]["out"].astype(np.float32)
    return out


if __name__ == "__main__":
    pass


# revision 62
# speedup vs baseline: 1.0205x; 1.0205x over previous
"""GQA attention kernel for 8 TRN2 NeuronCores (tensor-parallel over heads).

Problem: B=2, S=2048, D=2048, HQ=32, HKV=8, HD=64, ALiBi + additive mask,
softmax, out-projection.  Each core owns 4 q-heads (= 1 kv head); each core
computes a full-shape partial of the output (its heads' contribution through
wo), and the host sums the 8 partials.

v2 layout strategy (per core):
  - projections run as fp8e4 DoubleRow matmuls on host-prepared (hi, lo)
    residual pairs of x and wqkv (hi*hi + cross terms), which the cost
    model rates at 0.5 cycles/row; weights are pre-scaled by 1024 so the
    lo residuals stay in e4m3's normal range, undone during eviction.
  - everything else in bf16 (wo, v, exp(logits), attention outputs, DRAM
    output partial); psum stays f32.  ALiBi aug rows need f32 range
    (slope*m up to ~2e3), so the logits matmul runs f32r on f32 qaug/kaug
    whose data rows are written from the f32 projection psum.
  - logits computed TRANSPOSED: logitsT[n, m] = kaug.T @ qaug with the
    contraction dim augmented by 2 rows that add alibi slope*(n-m) and a
    per-query stabilizer -c[m] for free:
       kaug = [kT(64); n; 1]            (shared by all 4 heads)
       qaug_h = [qT_h(64); slope_h; -slope_h*m - c_h[m]]
  - attention is pipelined per 512-query chunk: logits -> exp(ACT, bf16
    out) with the AV matmuls emitted LAG chunk-steps later so the exp
    round-trip latency never blocks the in-order PE queue.  qk psum tiles
    rotate through a 4-deep ring; all other psum users (kv-projection,
    v-transpose, AV accumulators, out-projection) share a second 4-deep
    ring of banks, placed so no allocation ever waits on a slow eviction.
  - AV matmul uses vaug = [v | ones] so the ones column accumulates
    softmax denominators in psum row 64.  AV matmuls are column-trimmed
    to the causal region with per-diagonal-block stop flags.
  - normalization: denominator row is copied out of psum (DVE),
    reciprocal'd in place, partition-broadcast (Pool), and multiplied
    into the bf16 psum eviction (DVE).  Odd heads are DMA-shifted to
    partitions 64:127 so the o-projection reads one contiguous [128, m]
    stationary per head pair.
  - out-projection is split into per-128-query units and software-
    pipelined: units are interleaved into the NEXT attention/projection
    phase so the PE never waits on the normalize chain.
  - causal masks: dead logit tiles are skipped; diagonal-crossing tiles
    accumulate a precomputed [128,128] additive pattern on the PE itself
    (ident128.T @ mpat in bf16, 53ns) instead of a DVE pass.
"""

import sys

sys.path.insert(0, "/opt/trn_rl_repo")

import numpy as np

NEG = -1e9


# ---------------------------------------------------------------------------
# device program builder
# ---------------------------------------------------------------------------

def build_program(cfg):
    import concourse.bass as bass  # noqa: F401
    import concourse.mybir as mybir
    import concourse.tile as tile
    from concourse import bacc

    f32 = mybir.dt.float32
    f32r = mybir.dt.float32r
    bf16 = mybir.dt.bfloat16

    B, S, D = cfg["B"], cfg["S"], cfg["D"]
    HLOC, HD = cfg["HLOC"], cfg["HD"]
    MC = cfg["MC"]                    # m-chunk (<= 512, psum bank)
    MPAIR = 2 * MC                    # exp / AV / normalize granularity
    causal = cfg["causal"]
    generic_mask = cfg["generic_mask"]

    DQ = HLOC * HD                    # local q dims (256)
    DKV = 2 * HD                      # local kv dims (128)
    NKT = D // 128                    # contraction k-tiles for projections
    NNT = S // 128                    # n-tiles (keys)
    NMC = S // MC                     # m-chunks per b
    NPAIR = S // MPAIR                # m-pairs per b
    NHP = HLOC // 2                   # head pairs
    NEC = D // MC                     # out-proj e-chunks

    nc = bacc.Bacc("TRN2", target_bir_lowering=False, debug=False)

    fp8 = mybir.dt.float8e4
    WPAD = 512                        # wqkv cols padded for 512B dma elems
    # x and wqkv as fp8 (hi, lo) residual pairs for DoubleRow matmuls;
    # layouts [kt, slot, p, cols]; w slots (0=lo, 1=hi), x slots (0=hi, 1=lo)
    xT_d = nc.dram_tensor("xT2", [NKT, 2, 128, B, S], fp8,
                          kind="ExternalInput")
    wqkv_d = nc.dram_tensor("wqkv2", [NKT, 2, 128, WPAD], fp8,
                            kind="ExternalInput")
    wo_d = nc.dram_tensor("woT", [DQ, D], bf16, kind="ExternalInput")
    kaug_d = nc.dram_tensor("kaug_ext", [2, S], f32, kind="ExternalInput")
    qaug_d = nc.dram_tensor("qaug_ext", [HLOC, 2, S], f32, kind="ExternalInput")
    ident_d = nc.dram_tensor("ident", [64, 64], bf16, kind="ExternalInput")
    if causal:
        # mask pattern applied on the PE: qk += ident128.T @ mpat
        ident128_d = nc.dram_tensor("ident128", [128, 128], bf16,
                                    kind="ExternalInput")
        mpat_d = nc.dram_tensor("maskpat", [128, 128], bf16,
                                kind="ExternalInput")
    if generic_mask:
        maskT_d = nc.dram_tensor("maskT", [S, S], f32, kind="ExternalInput")
    out_d = nc.dram_tensor("out", [B, S, D], bf16, kind="ExternalOutput")

    def live(nt, mc):
        """is logitsT tile (keys nt*128.., queries mc*MC..) not fully masked"""
        if not causal:
            return True
        return nt * 128 <= mc * MC + MC - 1

    def crossing(nt, mc):
        """does the tile cross the causal diagonal (needs mask pattern)"""
        if not causal:
            return False
        return live(nt, mc) and nt * 128 + 127 > mc * MC

    with tile.TileContext(nc) as tc:
        with tc.tile_pool(name="res", bufs=1) as res, \
             tc.tile_pool(name="dbl", bufs=2) as dbl, \
             tc.tile_pool(name="sbp", bufs=3) as sbp, \
             tc.tile_pool(name="ps", bufs=1, space="PSUM") as ps:

            # ---- resident weights ----------------------------------------
            # wqkv quarters go on the SP queue (needed by the first matmul);
            # everything else loads via the ACT queue so the first xt DMA
            # isn't stuck behind resident loads on the in-order SP queue.
            wqkv_sb = res.tile([128, NKT, 2, WPAD], fp8, tag="wqkv")
            qtr = NKT // 4

            def _wqkv_quarter(qi):
                nc.sync.dma_start(
                    wqkv_sb[:, qi * qtr:(qi + 1) * qtr, :, :],
                    wqkv_d.ap()[qi * qtr:(qi + 1) * qtr]
                    .rearrange("kt two p q -> p kt two q"))

            # quarter 0 now; 1-3 deferred until after the first xt DMA so the
            # first projection matmul isn't stuck behind them on DMA_ENGINES
            _wqkv_quarter(0)
            deferred = [lambda qi=qi: _wqkv_quarter(qi) for qi in range(1, 4)]
            wo_sb = res.tile([128, NHP, D], bf16, tag="wo")
            ident_sb = res.tile([64, 64], bf16, tag="ident")
            if causal:
                ident128_sb = res.tile([128, 128], bf16, tag="ident128")
                mpat_sb = res.tile([128, 128], bf16, tag="mpat")

            def _load_misc():
                nc.scalar.dma_start(
                    wo_sb[:],
                    wo_d.ap()[:].rearrange("(hp p) e -> p hp e", p=128))
                nc.scalar.dma_start(ident_sb[:], ident_d.ap()[:])
                if causal:
                    nc.scalar.dma_start(ident128_sb[:], ident128_d.ap()[:])
                    nc.scalar.dma_start(mpat_sb[:], mpat_d.ap()[:])

            deferred.append(lambda: _load_misc())

            # per-b double-buffered activations (allocated inside the b loop)
            state = {}
            alt = {"i": 0}  # DVE/Pool alternation for mask adds + oproj evicts

            def proj_mc(b, mc):
                """projections for m-chunk mc of batch b"""
                kaug, qaug, vt = state["kaug"], state["qaug"], state["vt"]
                mco = mc * MC
                qp = [ps.tile([128, MC], f32, tag="qk", bufs=4,
                              name=f"qp{hp}") for hp in range(NHP)]
                kvp = ps.tile([128, MC], f32, tag="ps4", bufs=4, name="kvp")
                KQ = 4  # k-tiles per xt DMA
                DR = mybir.MatmulPerfMode.DoubleRow
                for ktq in range(NKT // KQ):
                    xt = sbp.tile([128, KQ, 2, MC], fp8, tag="xt", bufs=6)
                    nc.sync.dma_start(
                        xt[:], xT_d.ap()[ktq * KQ:(ktq + 1) * KQ, :, :,
                                         b, mco:mco + MC]
                        .rearrange("kt two p m -> p kt two m"))
                    while deferred:
                        deferred.pop(0)()
                    st = (ktq == 0)
                    sp = (ktq == NKT // KQ - 1)
                    groups = [(qp[0], 0), (qp[1], 128), (kvp, DQ)]
                    for dst, g0 in groups:
                        csl = slice(g0, g0 + 128) if g0 < DQ                             else slice(DQ, DQ + DKV)
                        # hi*hi over kt pairs
                        for kp in range(KQ // 2):
                            nc.tensor.matmul(
                                dst[:],
                                wqkv_sb[:, ktq * KQ + 2 * kp:
                                        ktq * KQ + 2 * kp + 2, 1, csl],
                                xt[:, 2 * kp:2 * kp + 2, 0, :],
                                start=st and kp == 0, stop=False,
                                perf_mode=DR)
                        # cross terms (w_lo x_hi + w_hi x_lo) per kt
                        for kq in range(KQ):
                            nc.tensor.matmul(
                                dst[:],
                                wqkv_sb[:, ktq * KQ + kq, :, csl],
                                xt[:, kq, :, :],
                                start=False, stop=sp and kq == KQ - 1,
                                perf_mode=DR)
                # evictions, spread across DVE/ACT so qp frees fast
                # (GPSIMD cannot access PSUM)
                WS = 1.0 / 1024.0  # undo the fp8 weight scaling
                for hp in range(NHP):
                    # even head of the pair: psum rows 0:64 -> qaug rows 0:64
                    nc.vector.tensor_scalar_mul(
                        qaug[2 * hp][0:64, mco:mco + MC], qp[hp][0:64, :], WS)
                    # odd head: rows 64:128, engine-copy then DMA shift
                    qtmp = sbp.tile([128, MC], f32r, tag="qtmp", bufs=4,
                                    name="qtmp")
                    nc.vector.tensor_scalar_mul(qtmp[64:128, :],
                                                qp[hp][64:128, :], WS)
                    nc.sync.dma_start(qaug[2 * hp + 1][0:64, mco:mco + MC],
                                      qtmp[64:128, :])
                nc.vector.tensor_scalar_mul(kaug[0:64, mco:mco + MC],
                                            kvp[0:64, :], WS)
                vtmp = sbp.tile([128, MC], bf16, tag="vtmp", bufs=2,
                                name="vtmp")
                nc.scalar.activation(vtmp[64:128, :], kvp[64:128, :],
                                     mybir.ActivationFunctionType.Copy,
                                     scale=WS)
                nc.sync.dma_start(vt[0:64, mco:mco + MC], vtmp[64:128, :])

            def vtrans(b):
                """transpose vT -> v (vaug), groups of 8 n-tiles per psum"""
                vt, vaug = state["vt"], state["vaug"]
                for g in range((NNT + 7) // 8):
                    nts = range(g * 8, min((g + 1) * 8, NNT))
                    vtp = ps.tile([128, 512], bf16, tag="ps4", bufs=4,
                                  name="vtp")
                    for j, nt in enumerate(nts):
                        nc.tensor.transpose(
                            vtp[:, j * 64:(j + 1) * 64],
                            vt[0:64, nt * 128:(nt + 1) * 128], ident_sb[:])
                    nc.vector.tensor_copy(vaug[:, nts.start:nts.stop, 0:HD],
                                          vtp[:, 0:64 * len(nts)].rearrange(
                                              "p (t d) -> p t d", d=64))

            def attn_head(b, pair, h):
                kaug, qaug, vaug = state["kaug"], state["qaug"], state["vaug"]
                OT = state["OT"]
                hp, odd = h // 2, h % 2
                av = [ps.tile([128, MC], f32, tag="ps4", bufs=4,
                              name=f"av{c}") for c in range(2)]
                nlive = [nt for nt in range(NNT)
                         if live(nt, 2 * pair) or live(nt, 2 * pair + 1)]
                last_nt = nlive[-1]

                def emit_av(nt, c, pt):
                    st = (nt == 0)
                    mc = 2 * pair + c
                    if causal:
                        # columns whose diagonal (last) tile is nt
                        sl = max(0, nt * 128 - mc * MC)
                        sh = min(MC, nt * 128 + 128 - mc * MC)
                        if sh > sl:
                            nc.tensor.matmul(
                                av[c][0:HD + 1, sl:sh],
                                vaug[:, nt, :], pt[:, sl:sh],
                                start=st, stop=True,
                                skip_group_check=True)
                            if sh < MC:
                                nc.tensor.matmul(
                                    av[c][0:HD + 1, sh:MC],
                                    vaug[:, nt, :], pt[:, sh:MC],
                                    start=st, stop=False,
                                    skip_group_check=True)
                        else:
                            nc.tensor.matmul(
                                av[c][0:HD + 1, :], vaug[:, nt, :], pt[:],
                                start=st, stop=False,
                                skip_group_check=True)
                    else:
                        nc.tensor.matmul(
                            av[c][0:HD + 1, :], vaug[:, nt, :], pt[:],
                            start=st, stop=(nt == last_nt))

                # software pipeline: AV runs LAG chunk-steps behind
                # logits/exp so the exp round-trip latency never blocks the
                # in-order PE queue
                LAG = 6
                fifo = []
                for nt in nlive:
                    for c in range(2):
                        mc = 2 * pair + c
                        if not live(nt, mc):
                            continue
                        o = max(0, nt * 128 - mc * MC) if causal else 0
                        qk = ps.tile([128, MC], f32, tag="qk", bufs=4,
                                     name="qk")
                        pt = sbp.tile([128, MC], bf16, tag="pt", bufs=8,
                                      name="pt")
                        cross = crossing(nt, mc)
                        nc.tensor.matmul(
                            qk[:, o:MC],
                            kaug[:, nt * 128:(nt + 1) * 128],
                            qaug[h][:, mc * MC + o:(mc + 1) * MC],
                            start=True, stop=not cross,
                            skip_group_check=cross)
                        if generic_mask:
                            mtile = sbp.tile([128, MC], f32, tag="mt",
                                             name="mt")
                            nc.sync.dma_start(
                                mtile[:],
                                maskT_d.ap()[nt * 128:(nt + 1) * 128,
                                             mc * MC:(mc + 1) * MC])
                            nc.vector.tensor_add(qk[:], qk[:], mtile[:])
                        elif cross:
                            # accumulate the causal pattern on the PE
                            nc.tensor.matmul(
                                qk[:, o:o + 128], ident128_sb[:],
                                mpat_sb[:], start=False, stop=True,
                                skip_group_check=True)
                        nc.scalar.activation(
                            pt[:, o:MC], qk[:, o:MC],
                            mybir.ActivationFunctionType.Exp)
                        fifo.append((nt, c, pt))
                        if len(fifo) > LAG:
                            emit_av(*fifo.pop(0))
                for item in fifo:
                    emit_av(*item)
                # normalize per chunk (chunk 0's chain overlaps chunk 1's
                # remaining AV matmuls): denom row -> broadcast -> divide
                # folded into the bf16 psum eviction
                qdiv = None
                if odd:
                    qdiv = sbp.tile([64, MPAIR], bf16, tag="qdiv", bufs=2,
                                    name="qdiv")
                for c in range(2):
                    cs = slice(c * MC, (c + 1) * MC)
                    srow = sbp.tile([1, MC], f32, tag="srow", bufs=6,
                                    name="srow")
                    nc.vector.tensor_copy(srow[0:1, :], av[c][HD:HD + 1, :])
                    nc.vector.reciprocal(srow[0:1, :], srow[0:1, :])
                    rbc = sbp.tile([128, MC], f32, tag="rbc", bufs=6,
                                   name="rbc")
                    nc.gpsimd.partition_broadcast(rbc[:], srow[0:1, :])
                    dst = qdiv[0:64, cs] if odd else OT[0:64, hp, cs]
                    nc.vector.tensor_mul(dst, av[c][0:64, :], rbc[0:64, :])
                if odd:
                    nc.sync.dma_start(OT[64:128, hp, :], qdiv[0:64, :])

            def oproj_mt(b, pair, OT, mtl, on_act):
                """out-projection for one 128-query tile of a pair; evicts on
                ACT during proj-phase drains (DVE is the proj bottleneck)"""
                mt = pair * (MPAIR // 128) + mtl
                ob = sbp.tile([128, D], bf16, tag="ob", bufs=2, name="ob")
                for ec in range(NEC):
                    op = ps.tile([128, MC], f32, tag="ps4", bufs=4, name="op")
                    for hp in range(NHP):
                        nc.tensor.matmul(
                            op[:],
                            OT[:, hp, mtl * 128:(mtl + 1) * 128],
                            wo_sb[:, hp, ec * MC:(ec + 1) * MC],
                            start=(hp == 0), stop=(hp == NHP - 1))
                    if on_act:
                        nc.scalar.activation(
                            ob[:, ec * MC:(ec + 1) * MC], op[:],
                            mybir.ActivationFunctionType.Copy)
                    else:
                        nc.vector.tensor_copy(ob[:, ec * MC:(ec + 1) * MC],
                                              op[:])
                nc.sync.dma_start(
                    out_d.ap()[b, mt * 128:(mt + 1) * 128, :], ob[:])

            # ---- schedule: proj/attention with interleaved out-proj ------
            pending = []

            def drain(k, on_act=False):
                for _ in range(min(k, len(pending))):
                    pending.pop(0)(on_act)

            for _rep in range(cfg.get("reps", 1)):
                for b in range(B):
                    state["kaug"] = dbl.tile([66, S], f32r, tag="kaug",
                                             bufs=2, name="kaug")
                    nc.scalar.dma_start(state["kaug"][64:66, :],
                                        kaug_d.ap()[:].bitcast(f32r))
                    qaug = []
                    for h in range(HLOC):
                        t = dbl.tile([66, S], f32r, tag=f"qaug{h}", bufs=2,
                                     name=f"qaug{h}")
                        nc.scalar.dma_start(t[64:66, :],
                                            qaug_d.ap()[h].bitcast(f32r))
                        qaug.append(t)
                    state["qaug"] = qaug
                    state["vt"] = dbl.tile([64, S], bf16, tag="vt", bufs=2,
                                           name="vt")
                    state["vaug"] = dbl.tile([128, NNT, HD + 1], bf16,
                                             tag="vaug", bufs=2, name="vaug")
                    nc.vector.memset(state["vaug"][:], 1.0)

                    for mc in range(NMC):
                        proj_mc(b, mc)
                    vtrans(b)
                    for pair in range(NPAIR):
                        OT = dbl.tile([128, NHP, MPAIR], bf16, tag="OT",
                                      bufs=3, name="OT")
                        state["OT"] = OT
                        # drain the out-proj filler where the PE has slack:
                        # pair-1 heads are ACT-bound (wide causal span),
                        # pair-0 heads less so; the PE-bound proj phase
                        # gets none.
                        dk = (0, 1, 2, 2) if pair == 0 else (2, 3, 3, 3)
                        # h3 before h2: the pair's final OT write is then a
                        # plain DVE multiply, not h3's slower DMA shift, so
                        # the tail out-proj starts ~2.5us sooner
                        for i, h in enumerate((0, 1, 3, 2)):
                            attn_head(b, pair, h)
                            drain(dk[i])
                        for mtl in range(MPAIR // 128):
                            pending.append(
                                lambda on_act, b=b, pair=pair, OT=OT,
                                mtl=mtl: oproj_mt(b, pair, OT, mtl, on_act))
                drain(len(pending))

    nc.compile()
    return nc


# ---------------------------------------------------------------------------
# host side
# ---------------------------------------------------------------------------

def _analyze_mask(mask2d, S):
    """classify mask; return (causal, zeros, n_lo, n_hi)"""
    masked = mask2d < -1e8
    if not masked.any():
        return False, True, np.zeros(S, np.int64), np.full(S, S - 1, np.int64)
    tri = np.triu(np.ones((S, S), bool), 1)
    if (masked == tri).all() and (mask2d[~masked] == 0).all():
        return True, False, np.zeros(S, np.int64), np.arange(S)
    allowed = ~masked
    # guard fully-masked rows (keep index 0; softmax row is garbage anyway)
    any_allowed = allowed.any(axis=1)
    idx = np.arange(S)[None, :]
    n_hi = np.where(any_allowed, np.where(allowed, idx, -1).max(axis=1), 0)
    n_lo = np.where(any_allowed, np.where(allowed, idx, S).min(axis=1), 0)
    return False, False, n_lo, n_hi


def _bf16(a):
    import ml_dtypes
    return np.ascontiguousarray(a).astype(ml_dtypes.bfloat16)


def _make_inputs_for_core(core, x, wq, wk, wv, wo, slopes, mask, cfg):
    B, S, D, HLOC, HD = cfg["B"], cfg["S"], cfg["D"], cfg["HLOC"], cfg["HD"]
    h0 = core * HLOC
    kv = core  # one kv head per core
    scale = 1.0 / np.sqrt(HD)

    import ml_dtypes
    FP8 = ml_dtypes.float8_e4m3
    NKT = D // 128
    DQ, DKV = HLOC * HD, 2 * HD
    WPAD = 512

    def _fp8_pair(a):
        hi = a.astype(FP8)
        lo = (a - hi.astype(np.float32)).astype(FP8)
        return hi, lo

    xT = np.ascontiguousarray(x.transpose(2, 0, 1))                 # [D,B,S]
    x_hi, x_lo = _fp8_pair(xT.reshape(NKT, 128, B, S))
    xT2 = np.stack([x_hi, x_lo], axis=1)                   # [kt,2,p,B,S]
    wqkvT = np.concatenate(
        [wq[h0 * HD:(h0 + HLOC) * HD] * scale,
         wk[kv * HD:(kv + 1) * HD],
         wv[kv * HD:(kv + 1) * HD]], axis=0).T                      # [D,384]
    wpad = np.zeros((D, WPAD), np.float32)
    wpad[:, :DQ + DKV] = wqkvT * 1024.0
    w_hi, w_lo = _fp8_pair(wpad.reshape(NKT, 128, WPAD))
    wqkv2 = np.stack([w_lo, w_hi], axis=1)                 # [kt,2,p,512]
    woT = np.ascontiguousarray(wo[:, h0 * HD:(h0 + HLOC) * HD].T)   # [DQ,D]

    n = np.arange(S, dtype=np.float32)
    kaug_ext = np.stack([n, np.ones(S, np.float32)])                # [2,S]

    qaug_ext = np.zeros((HLOC, 2, S), np.float32)
    for i in range(HLOC):
        sl = float(slopes[h0 + i])
        # stabilizer c[m] = max over allowed n of slope*(n-m), clipped >= 0
        c = np.maximum(0.0, np.maximum(sl * (cfg["n_hi"] - n),
                                       sl * (cfg["n_lo"] - n)))
        qaug_ext[i, 0, :] = sl
        qaug_ext[i, 1, :] = -sl * n - c

    ident = np.eye(64, dtype=np.float32)

    ins = {"xT2": xT2, "wqkv2": wqkv2, "woT": _bf16(woT),
           "kaug_ext": kaug_ext, "qaug_ext": qaug_ext,
           "ident": _bf16(ident)}
    if cfg["causal"]:
        ii = np.arange(128)[:, None]
        jj = np.arange(128)[None, :]
        ins["maskpat"] = _bf16(np.where(ii > jj, NEG, 0.0))
        ins["ident128"] = _bf16(np.eye(128))
    if cfg["generic_mask"]:
        ins["maskT"] = np.ascontiguousarray(mask[0, 0].T)
    return ins


def kernel(x, wq, wk, wv, wo, slopes, mask):
    from concourse.bass_utils import run_bass_kernel_spmd

    x = np.asarray(x, dtype=np.float32)
    wq = np.asarray(wq, dtype=np.float32)
    wk = np.asarray(wk, dtype=np.float32)
    wv = np.asarray(wv, dtype=np.float32)
    wo = np.asarray(wo, dtype=np.float32)
    slopes = np.asarray(slopes, dtype=np.float32)
    mask = np.asarray(mask, dtype=np.float32)

    B, S, D = x.shape
    HQ = 32
    HD = D // HQ
    n_cores = 8
    HLOC = HQ // n_cores

    causal, zeros, n_lo, n_hi = _analyze_mask(mask[0, 0], S)
    cfg = dict(B=B, S=S, D=D, HLOC=HLOC, HD=HD, MC=512,
               causal=causal, generic_mask=not (causal or zeros),
               n_lo=n_lo, n_hi=n_hi)

    nc = build_program(cfg)
    in_maps = [_make_inputs_for_core(c, x, wq, wk, wv, wo, slopes, mask, cfg)
               for c in range(n_cores)]
    res = run_bass_kernel_spmd(nc, in_maps, core_ids=list(range(n_cores)))
    out = np.zeros((B, S, D), np.float32)
    for c in range(n_cores):
        out += res.results[c]["out"].astype(np.float32)
    return out


if __name__ == "__main__":
    pass


# revision 69
# speedup vs baseline: 1.0729x; 1.0514x over previous
"""GQA attention kernel for 8 TRN2 NeuronCores (tensor-parallel over heads).

Problem: B=2, S=2048, D=2048, HQ=32, HKV=8, HD=64, ALiBi + additive mask,
softmax, out-projection.  Each core owns 4 q-heads (= 1 kv head); each core
computes a full-shape partial of the output (its heads' contribution through
wo), and the host sums the 8 partials.

v2 layout strategy (per core):
  - projections run as fp8e4 DoubleRow matmuls on host-prepared (hi, lo)
    residual pairs of x and wqkv (hi*hi + cross terms), which the cost
    model rates at 0.5 cycles/row; weights are pre-scaled by 1024 so the
    lo residuals stay in e4m3's normal range, undone during eviction.
  - everything else in bf16 (wo, v, exp(logits), attention outputs, DRAM
    output partial); psum stays f32.  ALiBi aug rows need f32 range
    (slope*m up to ~2e3), so the logits matmul runs f32r on f32 qaug/kaug
    whose data rows are written from the f32 projection psum.
  - logits computed TRANSPOSED: logitsT[n, m] = kaug.T @ qaug with the
    contraction dim augmented by 2 rows that add alibi slope*(n-m) and a
    per-query stabilizer -c[m] for free:
       kaug = [kT(64); n; 1]            (shared by all 4 heads)
       qaug_h = [qT_h(64); slope_h; -slope_h*m - c_h[m]]
  - attention is pipelined per 512-query chunk: logits -> exp(ACT, bf16
    out) with the AV matmuls emitted LAG chunk-steps later so the exp
    round-trip latency never blocks the in-order PE queue.  qk psum tiles
    rotate through a 4-deep ring; all other psum users (kv-projection,
    v-transpose, AV accumulators, out-projection) share a second 4-deep
    ring of banks, placed so no allocation ever waits on a slow eviction.
  - AV matmul uses vaug = [v | ones] so the ones column accumulates
    softmax denominators in psum row 64.  AV matmuls are column-trimmed
    to the causal region with per-diagonal-block stop flags.
  - normalization: denominator row is copied out of psum (DVE),
    reciprocal'd in place, partition-broadcast (Pool), and multiplied
    into the bf16 psum eviction (DVE).  Odd heads are DMA-shifted to
    partitions 64:127 so the o-projection reads one contiguous [128, m]
    stationary per head pair.
  - out-projection is split into per-128-query units and software-
    pipelined: units are interleaved into the NEXT attention/projection
    phase so the PE never waits on the normalize chain.
  - causal masks: dead logit tiles are skipped; diagonal-crossing tiles
    accumulate a precomputed [128,128] additive pattern on the PE itself
    (ident128.T @ mpat in bf16, 53ns) instead of a DVE pass.
"""

import sys

sys.path.insert(0, "/opt/trn_rl_repo")

import numpy as np

NEG = -1e9


# ---------------------------------------------------------------------------
# device program builder
# ---------------------------------------------------------------------------

def build_program(cfg):
    import concourse.bass as bass  # noqa: F401
    import concourse.mybir as mybir
    import concourse.tile as tile
    from concourse import bacc

    f32 = mybir.dt.float32
    f32r = mybir.dt.float32r
    bf16 = mybir.dt.bfloat16

    B, S, D = cfg["B"], cfg["S"], cfg["D"]
    HLOC, HD = cfg["HLOC"], cfg["HD"]
    MC = cfg["MC"]                    # m-chunk (<= 512, psum bank)
    MPAIR = 2 * MC                    # exp / AV / normalize granularity
    causal = cfg["causal"]
    generic_mask = cfg["generic_mask"]

    DQ = HLOC * HD                    # local q dims (256)
    DKV = 2 * HD                      # local kv dims (128)
    NKT = D // 128                    # contraction k-tiles for projections
    NNT = S // 128                    # n-tiles (keys)
    NMC = S // MC                     # m-chunks per b
    NPAIR = S // MPAIR                # m-pairs per b
    NHP = HLOC // 2                   # head pairs
    NEC = D // MC                     # out-proj e-chunks

    nc = bacc.Bacc("TRN2", target_bir_lowering=False, debug=False)

    fp8 = mybir.dt.float8e4
    WPAD = 512                        # wqkv cols padded for 512B dma elems
    # x and wqkv as fp8 (hi, lo) residual pairs for DoubleRow matmuls;
    # layouts [kt, slot, p, cols]; w slots (0=lo, 1=hi), x slots (0=hi, 1=lo)
    xT_d = nc.dram_tensor("xT2", [NKT, 2, 128, B, S], fp8,
                          kind="ExternalInput")
    wqkv_d = nc.dram_tensor("wqkv2", [NKT, 2, 128, WPAD], fp8,
                            kind="ExternalInput")
    wo_d = nc.dram_tensor("woT", [DQ, D], bf16, kind="ExternalInput")
    kaug_d = nc.dram_tensor("kaug_ext", [2, S], f32, kind="ExternalInput")
    qaug_d = nc.dram_tensor("qaug_ext", [HLOC, 2, S], f32, kind="ExternalInput")
    ident_d = nc.dram_tensor("ident", [64, 64], bf16, kind="ExternalInput")
    if causal:
        # mask pattern applied on the PE: qk += ident128.T @ mpat
        ident128_d = nc.dram_tensor("ident128", [128, 128], bf16,
                                    kind="ExternalInput")
        mpat_d = nc.dram_tensor("maskpat", [128, 128], bf16,
                                kind="ExternalInput")
    if generic_mask:
        maskT_d = nc.dram_tensor("maskT", [S, S], f32, kind="ExternalInput")
    out_d = nc.dram_tensor("out", [B, S, D], bf16, kind="ExternalOutput")

    def live(nt, mc):
        """is logitsT tile (keys nt*128.., queries mc*MC..) not fully masked"""
        if not causal:
            return True
        return nt * 128 <= mc * MC + MC - 1

    def crossing(nt, mc):
        """does the tile cross the causal diagonal (needs mask pattern)"""
        if not causal:
            return False
        return live(nt, mc) and nt * 128 + 127 > mc * MC

    with tile.TileContext(nc) as tc:
        with tc.tile_pool(name="res", bufs=1) as res, \
             tc.tile_pool(name="dbl", bufs=2) as dbl, \
             tc.tile_pool(name="sbp", bufs=3) as sbp, \
             tc.tile_pool(name="ps", bufs=1, space="PSUM") as ps:

            # ---- resident weights ----------------------------------------
            # wqkv quarters go on the SP queue (needed by the first matmul);
            # everything else loads via the ACT queue so the first xt DMA
            # isn't stuck behind resident loads on the in-order SP queue.
            wqkv_sb = res.tile([128, NKT, 2, WPAD], fp8, tag="wqkv")
            qtr = NKT // 4

            def _wqkv_quarter(qi):
                nc.sync.dma_start(
                    wqkv_sb[:, qi * qtr:(qi + 1) * qtr, :, :],
                    wqkv_d.ap()[qi * qtr:(qi + 1) * qtr]
                    .rearrange("kt two p q -> p kt two q"))

            # quarter 0 now; 1-3 deferred until after the first xt DMA so the
            # first projection matmul isn't stuck behind them on DMA_ENGINES
            _wqkv_quarter(0)
            deferred = [lambda qi=qi: _wqkv_quarter(qi) for qi in range(1, 4)]
            wo_sb = res.tile([128, NHP, D], bf16, tag="wo")
            ident_sb = res.tile([64, 64], bf16, tag="ident")
            if causal:
                ident128_sb = res.tile([128, 128], bf16, tag="ident128")
                mpat_sb = res.tile([128, 128], bf16, tag="mpat")

            def _load_misc():
                nc.scalar.dma_start(
                    wo_sb[:],
                    wo_d.ap()[:].rearrange("(hp p) e -> p hp e", p=128))
                nc.scalar.dma_start(ident_sb[:], ident_d.ap()[:])
                if causal:
                    nc.scalar.dma_start(ident128_sb[:], ident128_d.ap()[:])
                    nc.scalar.dma_start(mpat_sb[:], mpat_d.ap()[:])

            deferred.append(lambda: _load_misc())

            # per-b double-buffered activations (allocated inside the b loop)
            state = {}
            alt = {"i": 0}  # DVE/Pool alternation for mask adds + oproj evicts

            def proj_mc(b, mc):
                """projections for m-chunk mc of batch b"""
                kaug, qaug, vt = state["kaug"], state["qaug"], state["vt"]
                mco = mc * MC
                qp = [ps.tile([128, MC], f32, tag="qk", bufs=4,
                              name=f"qp{hp}") for hp in range(NHP)]
                kvp = ps.tile([128, MC], f32, tag="ps4", bufs=4, name="kvp")
                KQ = 4  # k-tiles per xt DMA
                DR = mybir.MatmulPerfMode.DoubleRow
                for ktq in range(NKT // KQ):
                    xt = sbp.tile([128, KQ, 2, MC], fp8, tag="xt", bufs=5)
                    nc.sync.dma_start(
                        xt[:], xT_d.ap()[ktq * KQ:(ktq + 1) * KQ, :, :,
                                         b, mco:mco + MC]
                        .rearrange("kt two p m -> p kt two m"))
                    while deferred:
                        deferred.pop(0)()
                    st = (ktq == 0)
                    sp = (ktq == NKT // KQ - 1)
                    groups = [(qp[0], 0), (qp[1], 128), (kvp, DQ)]
                    for dst, g0 in groups:
                        csl = slice(g0, g0 + 128) if g0 < DQ                             else slice(DQ, DQ + DKV)
                        # hi*hi over kt pairs
                        for kp in range(KQ // 2):
                            nc.tensor.matmul(
                                dst[:],
                                wqkv_sb[:, ktq * KQ + 2 * kp:
                                        ktq * KQ + 2 * kp + 2, 1, csl],
                                xt[:, 2 * kp:2 * kp + 2, 0, :],
                                start=st and kp == 0, stop=False,
                                perf_mode=DR)
                        # cross terms (w_lo x_hi + w_hi x_lo) per kt
                        for kq in range(KQ):
                            nc.tensor.matmul(
                                dst[:],
                                wqkv_sb[:, ktq * KQ + kq, :, csl],
                                xt[:, kq, :, :],
                                start=False, stop=sp and kq == KQ - 1,
                                perf_mode=DR)
                # evictions, spread across DVE/ACT so qp frees fast
                # (GPSIMD cannot access PSUM)
                WS = 1.0 / 1024.0  # undo the fp8 weight scaling
                for hp in range(NHP):
                    # even head of the pair: psum rows 0:64 -> qaug rows 0:64
                    nc.vector.tensor_scalar_mul(
                        qaug[2 * hp][0:64, mco:mco + MC], qp[hp][0:64, :], WS)
                    # odd head: rows 64:128, engine-copy then DMA shift
                    qtmp = sbp.tile([128, MC], f32r, tag="qtmp", bufs=3,
                                    name="qtmp")
                    nc.vector.tensor_scalar_mul(qtmp[64:128, :],
                                                qp[hp][64:128, :], WS)
                    nc.sync.dma_start(qaug[2 * hp + 1][0:64, mco:mco + MC],
                                      qtmp[64:128, :])
                nc.vector.tensor_scalar_mul(kaug[0:64, mco:mco + MC],
                                            kvp[0:64, :], WS)
                vtmp = sbp.tile([128, MC], bf16, tag="vtmp", bufs=2,
                                name="vtmp")
                nc.scalar.activation(vtmp[64:128, :], kvp[64:128, :],
                                     mybir.ActivationFunctionType.Copy,
                                     scale=WS)
                nc.sync.dma_start(vt[0:64, mco:mco + MC], vtmp[64:128, :])

            def vtrans(b):
                """transpose vT -> v (vaug), groups of 8 n-tiles per psum"""
                vt, vaug = state["vt"], state["vaug"]
                for g in range((NNT + 7) // 8):
                    nts = range(g * 8, min((g + 1) * 8, NNT))
                    vtp = ps.tile([128, 512], bf16, tag="ps4", bufs=4,
                                  name="vtp")
                    for j, nt in enumerate(nts):
                        nc.tensor.transpose(
                            vtp[:, j * 64:(j + 1) * 64],
                            vt[0:64, nt * 128:(nt + 1) * 128], ident_sb[:])
                    nc.vector.tensor_copy(vaug[:, nts.start:nts.stop, 0:HD],
                                          vtp[:, 0:64 * len(nts)].rearrange(
                                              "p (t d) -> p t d", d=64))

            def attn_head(b, pair, h):
                kaug, qaug, vaug = state["kaug"], state["qaug"], state["vaug"]
                OT = state["OT"]
                hp, odd = h // 2, h % 2
                av = [ps.tile([128, MC], f32, tag="ps4", bufs=4,
                              name=f"av{c}") for c in range(2)]
                nlive = [nt for nt in range(NNT)
                         if live(nt, 2 * pair) or live(nt, 2 * pair + 1)]
                last_nt = nlive[-1]

                def emit_av(nt, c, pt):
                    st = (nt == 0)
                    mc = 2 * pair + c
                    if causal:
                        # columns whose diagonal (last) tile is nt
                        sl = max(0, nt * 128 - mc * MC)
                        sh = min(MC, nt * 128 + 128 - mc * MC)
                        if sh > sl:
                            nc.tensor.matmul(
                                av[c][0:HD + 1, sl:sh],
                                vaug[:, nt, :], pt[:, sl:sh],
                                start=st, stop=True,
                                skip_group_check=True)
                            if sh < MC:
                                nc.tensor.matmul(
                                    av[c][0:HD + 1, sh:MC],
                                    vaug[:, nt, :], pt[:, sh:MC],
                                    start=st, stop=False,
                                    skip_group_check=True)
                        else:
                            nc.tensor.matmul(
                                av[c][0:HD + 1, :], vaug[:, nt, :], pt[:],
                                start=st, stop=False,
                                skip_group_check=True)
                    else:
                        nc.tensor.matmul(
                            av[c][0:HD + 1, :], vaug[:, nt, :], pt[:],
                            start=st, stop=(nt == last_nt))

                # software pipeline: AV runs LAG chunk-steps behind
                # logits/exp so the exp round-trip latency never blocks the
                # in-order PE queue
                LAG = 6
                fifo = []
                for nt in nlive:
                    for c in range(2):
                        mc = 2 * pair + c
                        if not live(nt, mc):
                            continue
                        o = max(0, nt * 128 - mc * MC) if causal else 0
                        qk = ps.tile([128, MC], f32, tag="qk", bufs=4,
                                     name="qk")
                        pt = sbp.tile([128, MC], bf16, tag="pt", bufs=8,
                                      name="pt")
                        cross = crossing(nt, mc)
                        nc.tensor.matmul(
                            qk[:, o:MC],
                            kaug[:, nt * 128:(nt + 1) * 128],
                            qaug[h][:, mc * MC + o:(mc + 1) * MC],
                            start=True, stop=not cross,
                            skip_group_check=cross)
                        if generic_mask:
                            mtile = sbp.tile([128, MC], f32, tag="mt",
                                             name="mt")
                            nc.sync.dma_start(
                                mtile[:],
                                maskT_d.ap()[nt * 128:(nt + 1) * 128,
                                             mc * MC:(mc + 1) * MC])
                            nc.vector.tensor_add(qk[:], qk[:], mtile[:])
                        elif cross:
                            # accumulate the causal pattern on the PE
                            nc.tensor.matmul(
                                qk[:, o:o + 128], ident128_sb[:],
                                mpat_sb[:], start=False, stop=True,
                                skip_group_check=True)
                        nc.scalar.activation(
                            pt[:, o:MC], qk[:, o:MC],
                            mybir.ActivationFunctionType.Exp)
                        fifo.append((nt, c, pt))
                        if len(fifo) > LAG:
                            emit_av(*fifo.pop(0))
                for item in fifo:
                    emit_av(*item)
                # normalize per chunk (chunk 0's chain overlaps chunk 1's
                # remaining AV matmuls): denom row -> broadcast -> divide
                # folded into the bf16 psum eviction
                qdiv = None
                if odd:
                    qdiv = sbp.tile([64, MPAIR], bf16, tag="qdiv", bufs=2,
                                    name="qdiv")
                for c in range(2):
                    cs = slice(c * MC, (c + 1) * MC)
                    srow = sbp.tile([1, MC], f32, tag="srow", bufs=6,
                                    name="srow")
                    nc.vector.tensor_copy(srow[0:1, :], av[c][HD:HD + 1, :])
                    nc.vector.reciprocal(srow[0:1, :], srow[0:1, :])
                    rbc = sbp.tile([128, MC], f32, tag="rbc", bufs=5,
                                   name="rbc")
                    nc.gpsimd.partition_broadcast(rbc[:], srow[0:1, :])
                    dst = qdiv[0:64, cs] if odd else OT[0:64, hp, cs]
                    nc.vector.tensor_mul(dst, av[c][0:64, :], rbc[0:64, :])
                if odd:
                    nc.sync.dma_start(OT[64:128, hp, :], qdiv[0:64, :])

            def oproj_mt(b, pair, OT, mtl, on_act):
                """out-projection for one 128-query tile of a pair; evicts on
                ACT during proj-phase drains (DVE is the proj bottleneck)"""
                mt = pair * (MPAIR // 128) + mtl
                ob = sbp.tile([128, D], bf16, tag="ob", bufs=4, name="ob")
                for ec in range(NEC):
                    op = ps.tile([128, MC], f32, tag="ps4", bufs=4, name="op")
                    for hp in range(NHP):
                        nc.tensor.matmul(
                            op[:],
                            OT[:, hp, mtl * 128:(mtl + 1) * 128],
                            wo_sb[:, hp, ec * MC:(ec + 1) * MC],
                            start=(hp == 0), stop=(hp == NHP - 1))
                    use_act = on_act == "alt" and ec % 2 or on_act is True
                    if use_act:
                        nc.scalar.activation(
                            ob[:, ec * MC:(ec + 1) * MC], op[:],
                            mybir.ActivationFunctionType.Copy)
                    else:
                        nc.vector.tensor_copy(ob[:, ec * MC:(ec + 1) * MC],
                                              op[:])
                nc.sync.dma_start(
                    out_d.ap()[b, mt * 128:(mt + 1) * 128, :], ob[:])

            # ---- schedule: proj/attention with interleaved out-proj ------
            pending = []

            def drain(k, on_act=False):
                for _ in range(min(k, len(pending))):
                    pending.pop(0)(on_act)

            for _rep in range(cfg.get("reps", 1)):
                for b in range(B):
                    state["kaug"] = dbl.tile([66, S], f32r, tag="kaug",
                                             bufs=2, name="kaug")
                    nc.scalar.dma_start(state["kaug"][64:66, :],
                                        kaug_d.ap()[:].bitcast(f32r))
                    qaug = []
                    for h in range(HLOC):
                        t = dbl.tile([66, S], f32r, tag=f"qaug{h}", bufs=2,
                                     name=f"qaug{h}")
                        nc.scalar.dma_start(t[64:66, :],
                                            qaug_d.ap()[h].bitcast(f32r))
                        qaug.append(t)
                    state["qaug"] = qaug
                    state["vt"] = dbl.tile([64, S], bf16, tag="vt", bufs=2,
                                           name="vt")
                    state["vaug"] = dbl.tile([128, NNT, HD + 1], bf16,
                                             tag="vaug", bufs=2, name="vaug")
                    nc.vector.memset(state["vaug"][:], 1.0)

                    for mc in range(NMC):
                        proj_mc(b, mc)
                    vtrans(b)
                    for pair in range(NPAIR):
                        OT = dbl.tile([128, NHP, MPAIR], bf16, tag="OT",
                                      bufs=3, name="OT")
                        state["OT"] = OT
                        # drain the out-proj filler where the PE has slack:
                        # pair-1 heads are ACT-bound (wide causal span),
                        # pair-0 heads less so; the PE-bound proj phase
                        # gets none.
                        dk = (0, 1, 2, 2) if pair == 0 else (2, 3, 3, 3)
                        # h3 before h2: the pair's final OT write is then a
                        # plain DVE multiply, not h3's slower DMA shift, so
                        # the tail out-proj starts ~2.5us sooner
                        for i, h in enumerate((0, 1, 3, 2)):
                            attn_head(b, pair, h)
                            # the last drain of pair 1 executes during the
                            # next proj phase where ACT is idle
                            alt_ok = pair == NPAIR - 1 and i == HLOC - 1
                            drain(dk[i], on_act="alt" if alt_ok else False)
                        for mtl in range(MPAIR // 128):
                            pending.append(
                                lambda on_act, b=b, pair=pair, OT=OT,
                                mtl=mtl: oproj_mt(b, pair, OT, mtl, on_act))
                drain(len(pending), on_act="alt")

    nc.compile()
    return nc


# ---------------------------------------------------------------------------
# host side
# ---------------------------------------------------------------------------

def _analyze_mask(mask2d, S):
    """classify mask; return (causal, zeros, n_lo, n_hi)"""
    masked = mask2d < -1e8
    if not masked.any():
        return False, True, np.zeros(S, np.int64), np.full(S, S - 1, np.int64)
    tri = np.triu(np.ones((S, S), bool), 1)
    if (masked == tri).all() and (mask2d[~masked] == 0).all():
        return True, False, np.zeros(S, np.int64), np.arange(S)
    allowed = ~masked
    # guard fully-masked rows (keep index 0; softmax row is garbage anyway)
    any_allowed = allowed.any(axis=1)
    idx = np.arange(S)[None, :]
    n_hi = np.where(any_allowed, np.where(allowed, idx, -1).max(axis=1), 0)
    n_lo = np.where(any_allowed, np.where(allowed, idx, S).min(axis=1), 0)
    return False, False, n_lo, n_hi


def _bf16(a):
    import ml_dtypes
    return np.ascontiguousarray(a).astype(ml_dtypes.bfloat16)


def _make_inputs_for_core(core, x, wq, wk, wv, wo, slopes, mask, cfg):
    B, S, D, HLOC, HD = cfg["B"], cfg["S"], cfg["D"], cfg["HLOC"], cfg["HD"]
    h0 = core * HLOC
    kv = core  # one kv head per core
    scale = 1.0 / np.sqrt(HD)

    import ml_dtypes
    FP8 = ml_dtypes.float8_e4m3
    NKT = D // 128
    DQ, DKV = HLOC * HD, 2 * HD
    WPAD = 512

    def _fp8_pair(a):
        hi = a.astype(FP8)
        lo = (a - hi.astype(np.float32)).astype(FP8)
        return hi, lo

    xT = np.ascontiguousarray(x.transpose(2, 0, 1))                 # [D,B,S]
    x_hi, x_lo = _fp8_pair(xT.reshape(NKT, 128, B, S))
    xT2 = np.stack([x_hi, x_lo], axis=1)                   # [kt,2,p,B,S]
    wqkvT = np.concatenate(
        [wq[h0 * HD:(h0 + HLOC) * HD] * scale,
         wk[kv * HD:(kv + 1) * HD],
         wv[kv * HD:(kv + 1) * HD]], axis=0).T                      # [D,384]
    wpad = np.zeros((D, WPAD), np.float32)
    wpad[:, :DQ + DKV] = wqkvT * 1024.0
    w_hi, w_lo = _fp8_pair(wpad.reshape(NKT, 128, WPAD))
    wqkv2 = np.stack([w_lo, w_hi], axis=1)                 # [kt,2,p,512]
    woT = np.ascontiguousarray(wo[:, h0 * HD:(h0 + HLOC) * HD].T)   # [DQ,D]

    n = np.arange(S, dtype=np.float32)
    kaug_ext = np.stack([n, np.ones(S, np.float32)])                # [2,S]

    qaug_ext = np.zeros((HLOC, 2, S), np.float32)
    for i in range(HLOC):
        sl = float(slopes[h0 + i])
        # stabilizer c[m] = max over allowed n of slope*(n-m), clipped >= 0
        c = np.maximum(0.0, np.maximum(sl * (cfg["n_hi"] - n),
                                       sl * (cfg["n_lo"] - n)))
        qaug_ext[i, 0, :] = sl
        qaug_ext[i, 1, :] = -sl * n - c

    ident = np.eye(64, dtype=np.float32)

    ins = {"xT2": xT2, "wqkv2": wqkv2, "woT": _bf16(woT),
           "kaug_ext": kaug_ext, "qaug_ext": qaug_ext,
           "ident": _bf16(ident)}
    if cfg["causal"]:
        ii = np.arange(128)[:, None]
        jj = np.arange(128)[None, :]
        ins["maskpat"] = _bf16(np.where(ii > jj, NEG, 0.0))
        ins["ident128"] = _bf16(np.eye(128))
    if cfg["generic_mask"]:
        ins["maskT"] = np.ascontiguousarray(mask[0, 0].T)
    return ins


def kernel(x, wq, wk, wv, wo, slopes, mask):
    from concourse.bass_utils import run_bass_kernel_spmd

    x = np.asarray(x, dtype=np.float32)
    wq = np.asarray(wq, dtype=np.float32)
    wk = np.asarray(wk, dtype=np.float32)
    wv = np.asarray(wv, dtype=np.float32)
    wo = np.asarray(wo, dtype=np.float32)
    slopes = np.asarray(slopes, dtype=np.float32)
    mask = np.asarray(mask, dtype=np.float32)

    B, S, D = x.shape
    HQ = 32
    HD = D // HQ
    n_cores = 8
    HLOC = HQ // n_cores

    causal, zeros, n_lo, n_hi = _analyze_mask(mask[0, 0], S)
    cfg = dict(B=B, S=S, D=D, HLOC=HLOC, HD=HD, MC=512,
               causal=causal, generic_mask=not (causal or zeros),
               n_lo=n_lo, n_hi=n_hi)

    nc = build_program(cfg)
    in_maps = [_make_inputs_for_core(c, x, wq, wk, wv, wo, slopes, mask, cfg)
               for c in range(n_cores)]
    res = run_bass_kernel_spmd(nc, in_maps, core_ids=list(range(n_cores)))
    out = np.zeros((B, S, D), np.float32)
    for c in range(n_cores):
        out += res.results[c]["out"].astype(np.float32)
    return out


if __name__ == "__main__":
    pass


# revision 77
# speedup vs baseline: 1.0754x; 1.0023x over previous
"""GQA attention kernel for 8 TRN2 NeuronCores (tensor-parallel over heads).

Problem: B=2, S=2048, D=2048, HQ=32, HKV=8, HD=64, ALiBi + additive mask,
softmax, out-projection.  Each core owns 4 q-heads (= 1 kv head); each core
computes a full-shape partial of the output (its heads' contribution through
wo), and the host sums the 8 partials.

v2 layout strategy (per core):
  - projections run as fp8e4 DoubleRow matmuls on host-prepared (hi, lo)
    residual pairs of x and wqkv (hi*hi + cross terms), which the cost
    model rates at 0.5 cycles/row; weights are pre-scaled by 1024 so the
    lo residuals stay in e4m3's normal range, undone during eviction.
  - everything else in bf16 (wo, v, exp(logits), attention outputs, DRAM
    output partial); psum stays f32.  ALiBi aug rows need f32 range
    (slope*m up to ~2e3), so the logits matmul runs f32r on f32 qaug/kaug
    whose data rows are written from the f32 projection psum.
  - logits computed TRANSPOSED: logitsT[n, m] = kaug.T @ qaug with the
    contraction dim augmented by 2 rows that add alibi slope*(n-m) and a
    per-query stabilizer -c[m] for free:
       kaug = [kT(64); n; 1]            (shared by all 4 heads)
       qaug_h = [qT_h(64); slope_h; -slope_h*m - c_h[m]]
  - attention is pipelined per 512-query chunk: logits -> exp(ACT, bf16
    out) with the AV matmuls emitted LAG chunk-steps later so the exp
    round-trip latency never blocks the in-order PE queue.  qk psum tiles
    rotate through a 4-deep ring; all other psum users (kv-projection,
    v-transpose, AV accumulators, out-projection) share a second 4-deep
    ring of banks, placed so no allocation ever waits on a slow eviction.
  - AV matmul uses vaug = [v | ones] so the ones column accumulates
    softmax denominators in psum row 64.  AV matmuls are column-trimmed
    to the causal region with per-diagonal-block stop flags.
  - normalization: denominator row is copied out of psum (DVE),
    reciprocal'd in place, partition-broadcast (Pool), and multiplied
    into the bf16 psum eviction (DVE).  Odd heads are DMA-shifted to
    partitions 64:127 so the o-projection reads one contiguous [128, m]
    stationary per head pair.
  - out-projection is split into per-128-query units and software-
    pipelined: units are interleaved into the NEXT attention/projection
    phase so the PE never waits on the normalize chain.
  - causal masks: dead logit tiles are skipped; diagonal-crossing tiles
    accumulate a precomputed [128,128] additive pattern on the PE itself
    (ident128.T @ mpat in bf16, 53ns) instead of a DVE pass.
"""

import sys

sys.path.insert(0, "/opt/trn_rl_repo")

import numpy as np

NEG = -1e9


# ---------------------------------------------------------------------------
# device program builder
# ---------------------------------------------------------------------------

def build_program(cfg):
    import concourse.bass as bass  # noqa: F401
    import concourse.mybir as mybir
    import concourse.tile as tile
    from concourse import bacc

    f32 = mybir.dt.float32
    f32r = mybir.dt.float32r
    bf16 = mybir.dt.bfloat16

    B, S, D = cfg["B"], cfg["S"], cfg["D"]
    HLOC, HD = cfg["HLOC"], cfg["HD"]
    MC = cfg["MC"]                    # m-chunk (<= 512, psum bank)
    MPAIR = 2 * MC                    # exp / AV / normalize granularity
    causal = cfg["causal"]
    generic_mask = cfg["generic_mask"]

    DQ = HLOC * HD                    # local q dims (256)
    DKV = 2 * HD                      # local kv dims (128)
    NKT = D // 128                    # contraction k-tiles for projections
    NNT = S // 128                    # n-tiles (keys)
    NMC = S // MC                     # m-chunks per b
    NPAIR = S // MPAIR                # m-pairs per b
    NHP = HLOC // 2                   # head pairs
    NEC = D // MC                     # out-proj e-chunks

    nc = bacc.Bacc("TRN2", target_bir_lowering=False, debug=False)

    fp8 = mybir.dt.float8e4
    # x and wqkv as fp8 (hi, lo) residual pairs for DoubleRow matmuls;
    # w slots (0=lo, 1=hi), x slots (0=hi, 1=lo).  wqkv is laid
    # [kt, p, slot, q] so the contiguous DMA run is 2*384=768B (no pad).
    xT_d = nc.dram_tensor("xT2", [NKT, 2, 128, B, S], fp8,
                          kind="ExternalInput")
    wqkv_d = nc.dram_tensor("wqkv2", [NKT, 128, 2, DQ + DKV], fp8,
                            kind="ExternalInput")
    wo_d = nc.dram_tensor("woT", [DQ, D], bf16, kind="ExternalInput")
    kaug_d = nc.dram_tensor("kaug_ext", [2, S], f32, kind="ExternalInput")
    qaug_d = nc.dram_tensor("qaug_ext", [HLOC, 2, S], f32, kind="ExternalInput")
    ident_d = nc.dram_tensor("ident", [64, 64], bf16, kind="ExternalInput")
    if causal:
        # mask pattern applied on the PE: qk += ident128.T @ mpat
        ident128_d = nc.dram_tensor("ident128", [128, 128], bf16,
                                    kind="ExternalInput")
        mpat_d = nc.dram_tensor("maskpat", [128, 128], bf16,
                                kind="ExternalInput")
    if generic_mask:
        maskT_d = nc.dram_tensor("maskT", [S, S], f32, kind="ExternalInput")
    out_d = nc.dram_tensor("out", [B, S, D], bf16, kind="ExternalOutput")

    def live(nt, mc):
        """is logitsT tile (keys nt*128.., queries mc*MC..) not fully masked"""
        if not causal:
            return True
        return nt * 128 <= mc * MC + MC - 1

    def crossing(nt, mc):
        """does the tile cross the causal diagonal (needs mask pattern)"""
        if not causal:
            return False
        return live(nt, mc) and nt * 128 + 127 > mc * MC

    with tile.TileContext(nc) as tc:
        with tc.tile_pool(name="res", bufs=1) as res, \
             tc.tile_pool(name="dbl", bufs=2) as dbl, \
             tc.tile_pool(name="sbp", bufs=3) as sbp, \
             tc.tile_pool(name="ps", bufs=1, space="PSUM") as ps:

            # ---- resident weights ----------------------------------------
            # wqkv quarters go on the SP queue (needed by the first matmul);
            # everything else loads via the ACT queue so the first xt DMA
            # isn't stuck behind resident loads on the in-order SP queue.
            wqkv_sb = res.tile([128, NKT, 2, DQ + DKV], fp8, tag="wqkv")
            qtr = NKT // 4

            def _wqkv_quarter(qi):
                nc.sync.dma_start(
                    wqkv_sb[:, qi * qtr:(qi + 1) * qtr, :, :],
                    wqkv_d.ap()[qi * qtr:(qi + 1) * qtr]
                    .rearrange("kt p two q -> p kt two q"))

            # quarter 0 now; 1-3 deferred until after the first xt DMA so the
            # first projection matmul isn't stuck behind them on DMA_ENGINES
            _wqkv_quarter(0)
            deferred = [lambda qi=qi: _wqkv_quarter(qi) for qi in range(1, 4)]
            wo_sb = res.tile([128, NHP, D], bf16, tag="wo")
            ident_sb = res.tile([64, 64], bf16, tag="ident")
            if causal:
                ident128_sb = res.tile([128, 128], bf16, tag="ident128")
                mpat_sb = res.tile([128, 128], bf16, tag="mpat")

            def _load_misc():
                nc.scalar.dma_start(
                    wo_sb[:],
                    wo_d.ap()[:].rearrange("(hp p) e -> p hp e", p=128))
                nc.scalar.dma_start(ident_sb[:], ident_d.ap()[:])
                if causal:
                    nc.scalar.dma_start(ident128_sb[:], ident128_d.ap()[:])
                    nc.scalar.dma_start(mpat_sb[:], mpat_d.ap()[:])

            deferred.append(lambda: _load_misc())

            # per-b double-buffered activations (allocated inside the b loop)
            state = {}
            alt = {"i": 0}  # DVE/Pool alternation for mask adds + oproj evicts

            def proj_mc(b, mc):
                """projections for m-chunk mc of batch b"""
                kaug, qaug, vt = state["kaug"], state["qaug"], state["vt"]
                mco = mc * MC
                qp = [ps.tile([128, MC], f32, tag="qk", bufs=4,
                              name=f"qp{hp}") for hp in range(NHP)]
                kvp = ps.tile([128, MC], f32, tag="ps4", bufs=4, name="kvp")
                KQ = 4  # k-tiles per xt DMA
                DR = mybir.MatmulPerfMode.DoubleRow
                for ktq in range(NKT // KQ):
                    xt = sbp.tile([128, KQ, 2, MC], fp8, tag="xt", bufs=6)
                    nc.sync.dma_start(
                        xt[:], xT_d.ap()[ktq * KQ:(ktq + 1) * KQ, :, :,
                                         b, mco:mco + MC]
                        .rearrange("kt two p m -> p kt two m"))
                    while deferred:
                        deferred.pop(0)()
                    st = (ktq == 0)
                    sp = (ktq == NKT // KQ - 1)
                    groups = [(qp[0], 0), (qp[1], 128), (kvp, DQ)]
                    for dst, g0 in groups:
                        csl = slice(g0, g0 + 128) if g0 < DQ                             else slice(DQ, DQ + DKV)
                        # hi*hi over kt pairs
                        for kp in range(KQ // 2):
                            nc.tensor.matmul(
                                dst[:],
                                wqkv_sb[:, ktq * KQ + 2 * kp:
                                        ktq * KQ + 2 * kp + 2, 1, csl],
                                xt[:, 2 * kp:2 * kp + 2, 0, :],
                                start=st and kp == 0, stop=False,
                                perf_mode=DR)
                        # cross terms (w_lo x_hi + w_hi x_lo) per kt
                        for kq in range(KQ):
                            nc.tensor.matmul(
                                dst[:],
                                wqkv_sb[:, ktq * KQ + kq, :, csl],
                                xt[:, kq, :, :],
                                start=False, stop=sp and kq == KQ - 1,
                                perf_mode=DR)
                # evictions, spread across DVE/ACT so qp frees fast
                # (GPSIMD cannot access PSUM)
                WS = 1.0 / 1024.0  # undo the fp8 weight scaling
                for hp in range(NHP):
                    # even head of the pair: psum rows 0:64 -> qaug rows 0:64
                    nc.vector.tensor_scalar_mul(
                        qaug[2 * hp][0:64, mco:mco + MC], qp[hp][0:64, :], WS)
                    # odd head: rows 64:128, engine-copy then DMA shift
                    qtmp = sbp.tile([128, MC], f32r, tag="qtmp", bufs=3,
                                    name="qtmp")
                    nc.vector.tensor_scalar_mul(qtmp[64:128, :],
                                                qp[hp][64:128, :], WS)
                    nc.sync.dma_start(qaug[2 * hp + 1][0:64, mco:mco + MC],
                                      qtmp[64:128, :])
                nc.vector.tensor_scalar_mul(kaug[0:64, mco:mco + MC],
                                            kvp[0:64, :], WS)
                vtmp = sbp.tile([128, MC], bf16, tag="vtmp", bufs=2,
                                name="vtmp")
                nc.scalar.activation(vtmp[64:128, :], kvp[64:128, :],
                                     mybir.ActivationFunctionType.Copy,
                                     scale=WS)
                nc.sync.dma_start(vt[0:64, mco:mco + MC], vtmp[64:128, :])

            def vtrans(b):
                """transpose vT -> v (vaug), groups of 8 n-tiles per psum"""
                vt, vaug = state["vt"], state["vaug"]
                for g in range((NNT + 7) // 8):
                    nts = range(g * 8, min((g + 1) * 8, NNT))
                    vtp = ps.tile([128, 512], bf16, tag="ps4", bufs=4,
                                  name="vtp")
                    for j, nt in enumerate(nts):
                        nc.tensor.transpose(
                            vtp[:, j * 64:(j + 1) * 64],
                            vt[0:64, nt * 128:(nt + 1) * 128], ident_sb[:])
                    nc.vector.tensor_copy(vaug[:, nts.start:nts.stop, 0:HD],
                                          vtp[:, 0:64 * len(nts)].rearrange(
                                              "p (t d) -> p t d", d=64))

            def attn_head(b, pair, h):
                kaug, qaug, vaug = state["kaug"], state["qaug"], state["vaug"]
                OT = state["OT"]
                hp, odd = h // 2, h % 2
                av = [ps.tile([128, MC], f32, tag="ps4", bufs=4,
                              name=f"av{c}") for c in range(2)]
                nlive = [nt for nt in range(NNT)
                         if live(nt, 2 * pair) or live(nt, 2 * pair + 1)]
                last_nt = nlive[-1]

                def emit_av(nt, c, pt):
                    st = (nt == 0)
                    mc = 2 * pair + c
                    if causal:
                        # columns whose diagonal (last) tile is nt
                        sl = max(0, nt * 128 - mc * MC)
                        sh = min(MC, nt * 128 + 128 - mc * MC)
                        if sh > sl:
                            nc.tensor.matmul(
                                av[c][0:HD + 1, sl:sh],
                                vaug[:, nt, :], pt[:, sl:sh],
                                start=st, stop=True,
                                skip_group_check=True)
                            if sh < MC:
                                nc.tensor.matmul(
                                    av[c][0:HD + 1, sh:MC],
                                    vaug[:, nt, :], pt[:, sh:MC],
                                    start=st, stop=False,
                                    skip_group_check=True)
                        else:
                            nc.tensor.matmul(
                                av[c][0:HD + 1, :], vaug[:, nt, :], pt[:],
                                start=st, stop=False,
                                skip_group_check=True)
                    else:
                        nc.tensor.matmul(
                            av[c][0:HD + 1, :], vaug[:, nt, :], pt[:],
                            start=st, stop=(nt == last_nt))

                # software pipeline: AV runs LAG chunk-steps behind
                # logits/exp so the exp round-trip latency never blocks the
                # in-order PE queue
                LAG = 6
                fifo = []
                for nt in nlive:
                    for c in range(2):
                        mc = 2 * pair + c
                        if not live(nt, mc):
                            continue
                        o = max(0, nt * 128 - mc * MC) if causal else 0
                        qk = ps.tile([128, MC], f32, tag="qk", bufs=4,
                                     name="qk")
                        pt = sbp.tile([128, MC], bf16, tag="pt", bufs=8,
                                      name="pt")
                        cross = crossing(nt, mc)
                        nc.tensor.matmul(
                            qk[:, o:MC],
                            kaug[:, nt * 128:(nt + 1) * 128],
                            qaug[h][:, mc * MC + o:(mc + 1) * MC],
                            start=True, stop=not cross,
                            skip_group_check=cross)
                        if generic_mask:
                            mtile = sbp.tile([128, MC], f32, tag="mt",
                                             name="mt")
                            nc.sync.dma_start(
                                mtile[:],
                                maskT_d.ap()[nt * 128:(nt + 1) * 128,
                                             mc * MC:(mc + 1) * MC])
                            nc.vector.tensor_add(qk[:], qk[:], mtile[:])
                        elif cross:
                            # accumulate the causal pattern on the PE
                            nc.tensor.matmul(
                                qk[:, o:o + 128], ident128_sb[:],
                                mpat_sb[:], start=False, stop=True,
                                skip_group_check=True)
                        nc.scalar.activation(
                            pt[:, o:MC], qk[:, o:MC],
                            mybir.ActivationFunctionType.Exp)
                        fifo.append((nt, c, pt))
                        if len(fifo) > LAG:
                            emit_av(*fifo.pop(0))
                for item in fifo:
                    emit_av(*item)
                # normalize per chunk (chunk 0's chain overlaps chunk 1's
                # remaining AV matmuls): denom row -> broadcast -> divide
                # folded into the bf16 psum eviction
                qdiv = None
                if odd:
                    qdiv = sbp.tile([64, MPAIR], bf16, tag="qdiv", bufs=2,
                                    name="qdiv")
                for c in range(2):
                    cs = slice(c * MC, (c + 1) * MC)
                    srow = sbp.tile([1, MC], f32, tag="srow", bufs=6,
                                    name="srow")
                    nc.vector.tensor_copy(srow[0:1, :], av[c][HD:HD + 1, :])
                    nc.vector.reciprocal(srow[0:1, :], srow[0:1, :])
                    rbc = sbp.tile([128, MC], f32, tag="rbc", bufs=5,
                                   name="rbc")
                    nc.gpsimd.partition_broadcast(rbc[:], srow[0:1, :])
                    dst = qdiv[0:64, cs] if odd else OT[0:64, hp, cs]
                    nc.vector.tensor_mul(dst, av[c][0:64, :], rbc[0:64, :])
                if odd:
                    nc.sync.dma_start(OT[64:128, hp, :], qdiv[0:64, :])

            def oproj_mt(b, pair, OT, mtl, on_act):
                """out-projection for one 128-query tile of a pair; evicts on
                ACT during proj-phase drains (DVE is the proj bottleneck)"""
                mt = pair * (MPAIR // 128) + mtl
                ob = sbp.tile([128, D], bf16, tag="ob", bufs=4, name="ob")
                for ec in range(NEC):
                    op = ps.tile([128, MC], f32, tag="ps4", bufs=4, name="op")
                    for hp in range(NHP):
                        nc.tensor.matmul(
                            op[:],
                            OT[:, hp, mtl * 128:(mtl + 1) * 128],
                            wo_sb[:, hp, ec * MC:(ec + 1) * MC],
                            start=(hp == 0), stop=(hp == NHP - 1))
                    use_act = on_act == "alt" and ec % 2 or on_act is True
                    if use_act:
                        nc.scalar.activation(
                            ob[:, ec * MC:(ec + 1) * MC], op[:],
                            mybir.ActivationFunctionType.Copy)
                    else:
                        nc.vector.tensor_copy(ob[:, ec * MC:(ec + 1) * MC],
                                              op[:])
                nc.sync.dma_start(
                    out_d.ap()[b, mt * 128:(mt + 1) * 128, :], ob[:])

            # ---- schedule: proj/attention with interleaved out-proj ------
            pending = []

            def drain(k, on_act=False):
                for _ in range(min(k, len(pending))):
                    pending.pop(0)(on_act)

            for _rep in range(cfg.get("reps", 1)):
                for b in range(B):
                    state["kaug"] = dbl.tile([66, S], f32r, tag="kaug",
                                             bufs=2, name="kaug")
                    nc.scalar.dma_start(state["kaug"][64:66, :],
                                        kaug_d.ap()[:].bitcast(f32r))
                    qaug = []
                    for h in range(HLOC):
                        t = dbl.tile([66, S], f32r, tag=f"qaug{h}", bufs=2,
                                     name=f"qaug{h}")
                        nc.scalar.dma_start(t[64:66, :],
                                            qaug_d.ap()[h].bitcast(f32r))
                        qaug.append(t)
                    state["qaug"] = qaug
                    state["vt"] = dbl.tile([64, S], bf16, tag="vt", bufs=2,
                                           name="vt")
                    state["vaug"] = dbl.tile([128, NNT, HD + 1], bf16,
                                             tag="vaug", bufs=2, name="vaug")
                    nc.vector.memset(state["vaug"][:], 1.0)

                    for mc in range(NMC):
                        proj_mc(b, mc)
                    vtrans(b)
                    for pair in range(NPAIR):
                        OT = dbl.tile([128, NHP, MPAIR], bf16, tag="OT",
                                      bufs=3, name="OT")
                        state["OT"] = OT
                        # drain the out-proj filler where the PE has slack:
                        # pair-1 heads are ACT-bound (wide causal span),
                        # pair-0 heads less so; the PE-bound proj phase
                        # gets none.
                        dk = (0, 1, 2, 2) if pair == 0 else (2, 3, 3, 3)
                        # h3 before h2: the pair's final OT write is then a
                        # plain DVE multiply, not h3's slower DMA shift, so
                        # the tail out-proj starts ~2.5us sooner
                        for i, h in enumerate((0, 1, 3, 2)):
                            attn_head(b, pair, h)
                            # the last drain of pair 1 executes during the
                            # next proj phase where ACT is idle
                            alt_ok = pair == NPAIR - 1 and i == HLOC - 1
                            drain(dk[i], on_act="alt" if alt_ok else False)
                        for mtl in range(MPAIR // 128):
                            pending.append(
                                lambda on_act, b=b, pair=pair, OT=OT,
                                mtl=mtl: oproj_mt(b, pair, OT, mtl, on_act))
                drain(len(pending), on_act="alt")

    nc.compile()
    return nc


# ---------------------------------------------------------------------------
# host side
# ---------------------------------------------------------------------------

def _analyze_mask(mask2d, S):
    """classify mask; return (causal, zeros, n_lo, n_hi)"""
    masked = mask2d < -1e8
    if not masked.any():
        return False, True, np.zeros(S, np.int64), np.full(S, S - 1, np.int64)
    tri = np.triu(np.ones((S, S), bool), 1)
    if (masked == tri).all() and (mask2d[~masked] == 0).all():
        return True, False, np.zeros(S, np.int64), np.arange(S)
    allowed = ~masked
    # guard fully-masked rows (keep index 0; softmax row is garbage anyway)
    any_allowed = allowed.any(axis=1)
    idx = np.arange(S)[None, :]
    n_hi = np.where(any_allowed, np.where(allowed, idx, -1).max(axis=1), 0)
    n_lo = np.where(any_allowed, np.where(allowed, idx, S).min(axis=1), 0)
    return False, False, n_lo, n_hi


def _bf16(a):
    import ml_dtypes
    return np.ascontiguousarray(a).astype(ml_dtypes.bfloat16)


def _make_inputs_for_core(core, x, wq, wk, wv, wo, slopes, mask, cfg):
    B, S, D, HLOC, HD = cfg["B"], cfg["S"], cfg["D"], cfg["HLOC"], cfg["HD"]
    h0 = core * HLOC
    kv = core  # one kv head per core
    scale = 1.0 / np.sqrt(HD)

    import ml_dtypes
    FP8 = ml_dtypes.float8_e4m3
    NKT = D // 128
    DQ, DKV = HLOC * HD, 2 * HD
    def _fp8_pair(a):
        hi = a.astype(FP8)
        lo = (a - hi.astype(np.float32)).astype(FP8)
        return hi, lo

    xT = np.ascontiguousarray(x.transpose(2, 0, 1))                 # [D,B,S]
    x_hi, x_lo = _fp8_pair(xT.reshape(NKT, 128, B, S))
    xT2 = np.stack([x_hi, x_lo], axis=1)                   # [kt,2,p,B,S]
    wqkvT = np.concatenate(
        [wq[h0 * HD:(h0 + HLOC) * HD] * scale,
         wk[kv * HD:(kv + 1) * HD],
         wv[kv * HD:(kv + 1) * HD]], axis=0).T                      # [D,384]
    w_hi, w_lo = _fp8_pair((wqkvT * 1024.0).reshape(NKT, 128, DQ + DKV))
    wqkv2 = np.stack([w_lo, w_hi], axis=2)                 # [kt,p,2,384]
    woT = np.ascontiguousarray(wo[:, h0 * HD:(h0 + HLOC) * HD].T)   # [DQ,D]

    n = np.arange(S, dtype=np.float32)
    kaug_ext = np.stack([n, np.ones(S, np.float32)])                # [2,S]

    qaug_ext = np.zeros((HLOC, 2, S), np.float32)
    for i in range(HLOC):
        sl = float(slopes[h0 + i])
        # stabilizer c[m] = max over allowed n of slope*(n-m), clipped >= 0
        c = np.maximum(0.0, np.maximum(sl * (cfg["n_hi"] - n),
                                       sl * (cfg["n_lo"] - n)))
        qaug_ext[i, 0, :] = sl
        qaug_ext[i, 1, :] = -sl * n - c

    ident = np.eye(64, dtype=np.float32)

    ins = {"xT2": xT2, "wqkv2": wqkv2, "woT": _bf16(woT),
           "kaug_ext": kaug_ext, "qaug_ext": qaug_ext,
           "ident": _bf16(ident)}
    if cfg["causal"]:
        ii = np.arange(128)[:, None]
        jj = np.arange(128)[None, :]
        ins["maskpat"] = _bf16(np.where(ii > jj, NEG, 0.0))
        ins["ident128"] = _bf16(np.eye(128))
    if cfg["generic_mask"]:
        ins["maskT"] = np.ascontiguousarray(mask[0, 0].T)
    return ins


def kernel(x, wq, wk, wv, wo, slopes, mask):
    from concourse.bass_utils import run_bass_kernel_spmd

    x = np.asarray(x, dtype=np.float32)
    wq = np.asarray(wq, dtype=np.float32)
    wk = np.asarray(wk, dtype=np.float32)
    wv = np.asarray(wv, dtype=np.float32)
    wo = np.asarray(wo, dtype=np.float32)
    slopes = np.asarray(slopes, dtype=np.float32)
    mask = np.asarray(mask, dtype=np.float32)

    B, S, D = x.shape
    HQ = 32
    HD = D // HQ
    n_cores = 8
    HLOC = HQ // n_cores

    causal, zeros, n_lo, n_hi = _analyze_mask(mask[0, 0], S)
    cfg = dict(B=B, S=S, D=D, HLOC=HLOC, HD=HD, MC=512,
               causal=causal, generic_mask=not (causal or zeros),
               n_lo=n_lo, n_hi=n_hi)

    nc = build_program(cfg)
    in_maps = [_make_inputs_for_core(c, x, wq, wk, wv, wo, slopes, mask, cfg)
               for c in range(n_cores)]
    res = run_bass_kernel_spmd(nc, in_maps, core_ids=list(range(n_cores)))
    out = np.zeros((B, S, D), np.float32)
    for c in range(n_cores):
        out += res.results[c]["out"].astype(np.float32)
    return out


if __name__ == "__main__":
    pass


# revision 78
# speedup vs baseline: 1.0836x; 1.0076x over previous
"""GQA attention kernel for 8 TRN2 NeuronCores (tensor-parallel over heads).

Problem: B=2, S=2048, D=2048, HQ=32, HKV=8, HD=64, ALiBi + additive mask,
softmax, out-projection.  Each core owns 4 q-heads (= 1 kv head); each core
computes a full-shape partial of the output (its heads' contribution through
wo), and the host sums the 8 partials.

v2 layout strategy (per core):
  - projections run as fp8e4 DoubleRow matmuls on host-prepared (hi, lo)
    residual pairs of x and wqkv (hi*hi + cross terms), which the cost
    model rates at 0.5 cycles/row; weights are pre-scaled by 1024 so the
    lo residuals stay in e4m3's normal range, undone during eviction.
  - everything else in bf16 (wo, v, exp(logits), attention outputs, DRAM
    output partial); psum stays f32.  ALiBi aug rows need f32 range
    (slope*m up to ~2e3), so the logits matmul runs f32r on f32 qaug/kaug
    whose data rows are written from the f32 projection psum.
  - logits computed TRANSPOSED: logitsT[n, m] = kaug.T @ qaug with the
    contraction dim augmented by 2 rows that add alibi slope*(n-m) and a
    per-query stabilizer -c[m] for free:
       kaug = [kT(64); n; 1]            (shared by all 4 heads)
       qaug_h = [qT_h(64); slope_h; -slope_h*m - c_h[m]]
  - attention is pipelined per 512-query chunk: logits -> exp(ACT, bf16
    out) with the AV matmuls emitted LAG chunk-steps later so the exp
    round-trip latency never blocks the in-order PE queue.  qk psum tiles
    rotate through a 4-deep ring; all other psum users (kv-projection,
    v-transpose, AV accumulators, out-projection) share a second 4-deep
    ring of banks, placed so no allocation ever waits on a slow eviction.
  - AV matmul uses vaug = [v | ones] so the ones column accumulates
    softmax denominators in psum row 64.  AV matmuls are column-trimmed
    to the causal region with per-diagonal-block stop flags.
  - normalization: denominator row is copied out of psum (DVE),
    reciprocal'd in place, partition-broadcast (Pool), and multiplied
    into the bf16 psum eviction (DVE).  Odd heads are DMA-shifted to
    partitions 64:127 so the o-projection reads one contiguous [128, m]
    stationary per head pair.
  - out-projection is split into per-128-query units and software-
    pipelined: units are interleaved into the NEXT attention/projection
    phase so the PE never waits on the normalize chain.
  - causal masks: dead logit tiles are skipped; diagonal-crossing tiles
    accumulate a precomputed [128,128] additive pattern on the PE itself
    (ident128.T @ mpat in bf16, 53ns) instead of a DVE pass.
"""

import sys

sys.path.insert(0, "/opt/trn_rl_repo")

import numpy as np

NEG = -1e9


# ---------------------------------------------------------------------------
# device program builder
# ---------------------------------------------------------------------------

def build_program(cfg):
    import concourse.bass as bass  # noqa: F401
    import concourse.mybir as mybir
    import concourse.tile as tile
    from concourse import bacc

    f32 = mybir.dt.float32
    f32r = mybir.dt.float32r
    bf16 = mybir.dt.bfloat16

    B, S, D = cfg["B"], cfg["S"], cfg["D"]
    HLOC, HD = cfg["HLOC"], cfg["HD"]
    MC = cfg["MC"]                    # m-chunk (<= 512, psum bank)
    MPAIR = 2 * MC                    # exp / AV / normalize granularity
    causal = cfg["causal"]
    generic_mask = cfg["generic_mask"]

    DQ = HLOC * HD                    # local q dims (256)
    DKV = 2 * HD                      # local kv dims (128)
    NKT = D // 128                    # contraction k-tiles for projections
    NNT = S // 128                    # n-tiles (keys)
    NMC = S // MC                     # m-chunks per b
    NPAIR = S // MPAIR                # m-pairs per b
    NHP = HLOC // 2                   # head pairs
    NEC = D // MC                     # out-proj e-chunks

    nc = bacc.Bacc("TRN2", target_bir_lowering=False, debug=False)

    fp8 = mybir.dt.float8e4
    # x and wqkv as fp8 (hi, lo) residual pairs for DoubleRow matmuls;
    # w slots (0=lo, 1=hi), x slots (0=hi, 1=lo).  wqkv is laid
    # [kt, p, slot, q] so the contiguous DMA run is 2*384=768B (no pad).
    xT_d = nc.dram_tensor("xT2", [NKT, 2, 128, B, S], fp8,
                          kind="ExternalInput")
    wqkv_d = nc.dram_tensor("wqkv2", [NKT, 128, 2, DQ + DKV], fp8,
                            kind="ExternalInput")
    wo_d = nc.dram_tensor("woT", [DQ, D], bf16, kind="ExternalInput")
    kaug_d = nc.dram_tensor("kaug_ext", [2, S], f32, kind="ExternalInput")
    qaug_d = nc.dram_tensor("qaug_ext", [HLOC, 2, S], f32, kind="ExternalInput")
    ident_d = nc.dram_tensor("ident", [64, 64], bf16, kind="ExternalInput")
    if causal:
        # mask pattern applied on the PE: qk += ident128.T @ mpat
        ident128_d = nc.dram_tensor("ident128", [128, 128], bf16,
                                    kind="ExternalInput")
        mpat_d = nc.dram_tensor("maskpat", [128, 128], bf16,
                                kind="ExternalInput")
    if generic_mask:
        maskT_d = nc.dram_tensor("maskT", [S, S], f32, kind="ExternalInput")
    out_d = nc.dram_tensor("out", [B, S, D], bf16, kind="ExternalOutput")

    def live(nt, mc):
        """is logitsT tile (keys nt*128.., queries mc*MC..) not fully masked"""
        if not causal:
            return True
        return nt * 128 <= mc * MC + MC - 1

    def crossing(nt, mc):
        """does the tile cross the causal diagonal (needs mask pattern)"""
        if not causal:
            return False
        return live(nt, mc) and nt * 128 + 127 > mc * MC

    with tile.TileContext(nc) as tc:
        with tc.tile_pool(name="res", bufs=1) as res, \
             tc.tile_pool(name="dbl", bufs=2) as dbl, \
             tc.tile_pool(name="sbp", bufs=3) as sbp, \
             tc.tile_pool(name="ps", bufs=1, space="PSUM") as ps:

            # ---- resident weights ----------------------------------------
            # wqkv quarters go on the SP queue (needed by the first matmul);
            # everything else loads via the ACT queue so the first xt DMA
            # isn't stuck behind resident loads on the in-order SP queue.
            wqkv_sb = res.tile([128, NKT, 2, DQ + DKV], fp8, tag="wqkv")
            qtr = NKT // 4

            def _wqkv_quarter(qi):
                nc.sync.dma_start(
                    wqkv_sb[:, qi * qtr:(qi + 1) * qtr, :, :],
                    wqkv_d.ap()[qi * qtr:(qi + 1) * qtr]
                    .rearrange("kt p two q -> p kt two q"))

            # quarter 0 now; 1-3 deferred until after the first xt DMA so the
            # first projection matmul isn't stuck behind them on DMA_ENGINES
            _wqkv_quarter(0)
            deferred = [lambda qi=qi: _wqkv_quarter(qi) for qi in range(1, 4)]
            wo_sb = res.tile([128, NHP, D], bf16, tag="wo")
            ident_sb = res.tile([64, 64], bf16, tag="ident")
            if causal:
                ident128_sb = res.tile([128, 128], bf16, tag="ident128")
                mpat_sb = res.tile([128, 128], bf16, tag="mpat")

            def _load_misc():
                nc.scalar.dma_start(
                    wo_sb[:],
                    wo_d.ap()[:].rearrange("(hp p) e -> p hp e", p=128))
                nc.scalar.dma_start(ident_sb[:], ident_d.ap()[:])
                if causal:
                    nc.scalar.dma_start(ident128_sb[:], ident128_d.ap()[:])
                    nc.scalar.dma_start(mpat_sb[:], mpat_d.ap()[:])

            deferred.append(lambda: _load_misc())

            # per-b double-buffered activations (allocated inside the b loop)
            state = {}
            alt = {"i": 0}  # DVE/Pool alternation for mask adds + oproj evicts

            def proj_mc(b, mc):
                """projections for m-chunk mc of batch b"""
                kaug, qaug, vt = state["kaug"], state["qaug"], state["vt"]
                mco = mc * MC
                qp = [ps.tile([128, MC], f32, tag="qk", bufs=4,
                              name=f"qp{hp}") for hp in range(NHP)]
                kvp = ps.tile([128, MC], f32, tag="ps4", bufs=4, name="kvp")
                KQ = 4  # k-tiles per xt DMA
                DR = mybir.MatmulPerfMode.DoubleRow
                for ktq in range(NKT // KQ):
                    xt = sbp.tile([128, KQ, 2, MC], fp8, tag="xt", bufs=6)
                    nc.sync.dma_start(
                        xt[:], xT_d.ap()[ktq * KQ:(ktq + 1) * KQ, :, :,
                                         b, mco:mco + MC]
                        .rearrange("kt two p m -> p kt two m"))
                    while deferred:
                        deferred.pop(0)()
                    st = (ktq == 0)
                    sp = (ktq == NKT // KQ - 1)
                    groups = [(qp[0], 0), (qp[1], 128), (kvp, DQ)]
                    for dst, g0 in groups:
                        csl = slice(g0, g0 + 128) if g0 < DQ                             else slice(DQ, DQ + DKV)
                        # hi*hi over kt pairs
                        for kp in range(KQ // 2):
                            nc.tensor.matmul(
                                dst[:],
                                wqkv_sb[:, ktq * KQ + 2 * kp:
                                        ktq * KQ + 2 * kp + 2, 1, csl],
                                xt[:, 2 * kp:2 * kp + 2, 0, :],
                                start=st and kp == 0, stop=False,
                                perf_mode=DR)
                        # cross terms (w_lo x_hi + w_hi x_lo) per kt
                        for kq in range(KQ):
                            nc.tensor.matmul(
                                dst[:],
                                wqkv_sb[:, ktq * KQ + kq, :, csl],
                                xt[:, kq, :, :],
                                start=False, stop=sp and kq == KQ - 1,
                                perf_mode=DR)
                # evictions, spread across DVE/ACT so qp frees fast
                # (GPSIMD cannot access PSUM)
                WS = 1.0 / 1024.0  # undo the fp8 weight scaling
                for hp in range(NHP):
                    # even head of the pair: psum rows 0:64 -> qaug rows 0:64
                    nc.vector.tensor_scalar_mul(
                        qaug[2 * hp][0:64, mco:mco + MC], qp[hp][0:64, :], WS)
                    # odd head: rows 64:128, engine-copy then DMA shift
                    qtmp = sbp.tile([128, MC], f32r, tag="qtmp", bufs=3,
                                    name="qtmp")
                    nc.vector.tensor_scalar_mul(qtmp[64:128, :],
                                                qp[hp][64:128, :], WS)
                    nc.sync.dma_start(qaug[2 * hp + 1][0:64, mco:mco + MC],
                                      qtmp[64:128, :])
                nc.vector.tensor_scalar_mul(kaug[0:64, mco:mco + MC],
                                            kvp[0:64, :], WS)
                vtmp = sbp.tile([128, MC], bf16, tag="vtmp", bufs=2,
                                name="vtmp")
                nc.scalar.activation(vtmp[64:128, :], kvp[64:128, :],
                                     mybir.ActivationFunctionType.Copy,
                                     scale=WS)
                nc.sync.dma_start(vt[0:64, mco:mco + MC], vtmp[64:128, :])

            def vtrans(b):
                """transpose vT -> v (vaug), groups of 8 n-tiles per psum"""
                vt, vaug = state["vt"], state["vaug"]
                for g in range((NNT + 7) // 8):
                    nts = range(g * 8, min((g + 1) * 8, NNT))
                    vtp = ps.tile([128, 512], bf16, tag="ps4", bufs=4,
                                  name="vtp")
                    for j, nt in enumerate(nts):
                        nc.tensor.transpose(
                            vtp[:, j * 64:(j + 1) * 64],
                            vt[0:64, nt * 128:(nt + 1) * 128], ident_sb[:])
                    nc.vector.tensor_copy(vaug[:, nts.start:nts.stop, 0:HD],
                                          vtp[:, 0:64 * len(nts)].rearrange(
                                              "p (t d) -> p t d", d=64))

            def attn_head(b, pair, h):
                kaug, qaug, vaug = state["kaug"], state["qaug"], state["vaug"]
                OT = state["OT"]
                hp, odd = h // 2, h % 2
                av = [ps.tile([128, MC], f32, tag="ps4", bufs=4,
                              name=f"av{c}") for c in range(2)]
                nlive = [nt for nt in range(NNT)
                         if live(nt, 2 * pair) or live(nt, 2 * pair + 1)]
                last_nt = nlive[-1]

                def emit_av(nt, c, pt):
                    st = (nt == 0)
                    mc = 2 * pair + c
                    if causal:
                        # columns whose diagonal (last) tile is nt
                        sl = max(0, nt * 128 - mc * MC)
                        sh = min(MC, nt * 128 + 128 - mc * MC)
                        if sh > sl:
                            nc.tensor.matmul(
                                av[c][0:HD + 1, sl:sh],
                                vaug[:, nt, :], pt[:, sl:sh],
                                start=st, stop=True,
                                skip_group_check=True)
                            if sh < MC:
                                nc.tensor.matmul(
                                    av[c][0:HD + 1, sh:MC],
                                    vaug[:, nt, :], pt[:, sh:MC],
                                    start=st, stop=False,
                                    skip_group_check=True)
                        else:
                            nc.tensor.matmul(
                                av[c][0:HD + 1, :], vaug[:, nt, :], pt[:],
                                start=st, stop=False,
                                skip_group_check=True)
                    else:
                        nc.tensor.matmul(
                            av[c][0:HD + 1, :], vaug[:, nt, :], pt[:],
                            start=st, stop=(nt == last_nt))

                # software pipeline: AV runs LAG chunk-steps behind
                # logits/exp so the exp round-trip latency never blocks the
                # in-order PE queue
                LAG = 6
                fifo = []
                for nt in nlive:
                    for c in range(2):
                        mc = 2 * pair + c
                        if not live(nt, mc):
                            continue
                        o = max(0, nt * 128 - mc * MC) if causal else 0
                        qk = ps.tile([128, MC], f32, tag="qk", bufs=4,
                                     name="qk")
                        pt = sbp.tile([128, MC], bf16, tag="pt", bufs=8,
                                      name="pt")
                        cross = crossing(nt, mc)
                        nc.tensor.matmul(
                            qk[:, o:MC],
                            kaug[:, nt * 128:(nt + 1) * 128],
                            qaug[h][:, mc * MC + o:(mc + 1) * MC],
                            start=True, stop=not cross,
                            skip_group_check=cross)
                        if generic_mask:
                            mtile = sbp.tile([128, MC], f32, tag="mt",
                                             name="mt")
                            nc.sync.dma_start(
                                mtile[:],
                                maskT_d.ap()[nt * 128:(nt + 1) * 128,
                                             mc * MC:(mc + 1) * MC])
                            nc.vector.tensor_add(qk[:], qk[:], mtile[:])
                        elif cross:
                            # accumulate the causal pattern on the PE
                            nc.tensor.matmul(
                                qk[:, o:o + 128], ident128_sb[:],
                                mpat_sb[:], start=False, stop=True,
                                skip_group_check=True)
                        nc.scalar.activation(
                            pt[:, o:MC], qk[:, o:MC],
                            mybir.ActivationFunctionType.Exp)
                        fifo.append((nt, c, pt))
                        if len(fifo) > LAG:
                            emit_av(*fifo.pop(0))
                for item in fifo:
                    emit_av(*item)
                # normalize per chunk (chunk 0's chain overlaps chunk 1's
                # remaining AV matmuls): denom row -> broadcast -> divide
                # folded into the bf16 psum eviction
                qdiv = None
                if odd:
                    qdiv = sbp.tile([64, MPAIR], bf16, tag="qdiv", bufs=2,
                                    name="qdiv")
                for c in range(2):
                    cs = slice(c * MC, (c + 1) * MC)
                    srow = sbp.tile([1, MC], f32, tag="srow", bufs=6,
                                    name="srow")
                    nc.vector.reciprocal(srow[0:1, :], av[c][HD:HD + 1, :])
                    rbc = sbp.tile([128, MC], f32, tag="rbc", bufs=5,
                                   name="rbc")
                    nc.gpsimd.partition_broadcast(rbc[:], srow[0:1, :])
                    dst = qdiv[0:64, cs] if odd else OT[0:64, hp, cs]
                    nc.vector.tensor_mul(dst, av[c][0:64, :], rbc[0:64, :])
                if odd:
                    nc.sync.dma_start(OT[64:128, hp, :], qdiv[0:64, :])

            def oproj_mt(b, pair, OT, mtl, on_act):
                """out-projection for one 128-query tile of a pair; evicts on
                ACT during proj-phase drains (DVE is the proj bottleneck)"""
                mt = pair * (MPAIR // 128) + mtl
                ob = sbp.tile([128, D], bf16, tag="ob", bufs=4, name="ob")
                for ec in range(NEC):
                    op = ps.tile([128, MC], f32, tag="ps4", bufs=4, name="op")
                    for hp in range(NHP):
                        nc.tensor.matmul(
                            op[:],
                            OT[:, hp, mtl * 128:(mtl + 1) * 128],
                            wo_sb[:, hp, ec * MC:(ec + 1) * MC],
                            start=(hp == 0), stop=(hp == NHP - 1))
                    use_act = on_act == "alt" and ec % 2 or on_act is True
                    if use_act:
                        nc.scalar.activation(
                            ob[:, ec * MC:(ec + 1) * MC], op[:],
                            mybir.ActivationFunctionType.Copy)
                    else:
                        nc.vector.tensor_copy(ob[:, ec * MC:(ec + 1) * MC],
                                              op[:])
                nc.sync.dma_start(
                    out_d.ap()[b, mt * 128:(mt + 1) * 128, :], ob[:])

            # ---- schedule: proj/attention with interleaved out-proj ------
            pending = []

            def drain(k, on_act=False):
                for _ in range(min(k, len(pending))):
                    pending.pop(0)(on_act)

            for _rep in range(cfg.get("reps", 1)):
                for b in range(B):
                    state["kaug"] = dbl.tile([66, S], f32r, tag="kaug",
                                             bufs=2, name="kaug")
                    nc.scalar.dma_start(state["kaug"][64:66, :],
                                        kaug_d.ap()[:].bitcast(f32r))
                    qaug = []
                    for h in range(HLOC):
                        t = dbl.tile([66, S], f32r, tag=f"qaug{h}", bufs=2,
                                     name=f"qaug{h}")
                        nc.scalar.dma_start(t[64:66, :],
                                            qaug_d.ap()[h].bitcast(f32r))
                        qaug.append(t)
                    state["qaug"] = qaug
                    state["vt"] = dbl.tile([64, S], bf16, tag="vt", bufs=2,
                                           name="vt")
                    state["vaug"] = dbl.tile([128, NNT, HD + 1], bf16,
                                             tag="vaug", bufs=2, name="vaug")
                    nc.vector.memset(state["vaug"][:], 1.0)

                    for mc in range(NMC):
                        proj_mc(b, mc)
                    vtrans(b)
                    for pair in range(NPAIR):
                        OT = dbl.tile([128, NHP, MPAIR], bf16, tag="OT",
                                      bufs=3, name="OT")
                        state["OT"] = OT
                        # drain the out-proj filler where the PE has slack:
                        # pair-1 heads are ACT-bound (wide causal span),
                        # pair-0 heads less so; the PE-bound proj phase
                        # gets none.
                        dk = (0, 1, 2, 2) if pair == 0 else (2, 3, 3, 3)
                        # h3 before h2: the pair's final OT write is then a
                        # plain DVE multiply, not h3's slower DMA shift, so
                        # the tail out-proj starts ~2.5us sooner
                        for i, h in enumerate((0, 1, 3, 2)):
                            attn_head(b, pair, h)
                            # the last drain of pair 1 executes during the
                            # next proj phase where ACT is idle
                            alt_ok = pair == NPAIR - 1 and i == HLOC - 1
                            drain(dk[i], on_act="alt" if alt_ok else False)
                        for mtl in range(MPAIR // 128):
                            pending.append(
                                lambda on_act, b=b, pair=pair, OT=OT,
                                mtl=mtl: oproj_mt(b, pair, OT, mtl, on_act))
                drain(len(pending), on_act="alt")

    nc.compile()
    return nc


# ---------------------------------------------------------------------------
# host side
# ---------------------------------------------------------------------------

def _analyze_mask(mask2d, S):
    """classify mask; return (causal, zeros, n_lo, n_hi)"""
    masked = mask2d < -1e8
    if not masked.any():
        return False, True, np.zeros(S, np.int64), np.full(S, S - 1, np.int64)
    tri = np.triu(np.ones((S, S), bool), 1)
    if (masked == tri).all() and (mask2d[~masked] == 0).all():
        return True, False, np.zeros(S, np.int64), np.arange(S)
    allowed = ~masked
    # guard fully-masked rows (keep index 0; softmax row is garbage anyway)
    any_allowed = allowed.any(axis=1)
    idx = np.arange(S)[None, :]
    n_hi = np.where(any_allowed, np.where(allowed, idx, -1).max(axis=1), 0)
    n_lo = np.where(any_allowed, np.where(allowed, idx, S).min(axis=1), 0)
    return False, False, n_lo, n_hi


def _bf16(a):
    import ml_dtypes
    return np.ascontiguousarray(a).astype(ml_dtypes.bfloat16)


def _make_inputs_for_core(core, x, wq, wk, wv, wo, slopes, mask, cfg):
    B, S, D, HLOC, HD = cfg["B"], cfg["S"], cfg["D"], cfg["HLOC"], cfg["HD"]
    h0 = core * HLOC
    kv = core  # one kv head per core
    scale = 1.0 / np.sqrt(HD)

    import ml_dtypes
    FP8 = ml_dtypes.float8_e4m3
    NKT = D // 128
    DQ, DKV = HLOC * HD, 2 * HD
    def _fp8_pair(a):
        hi = a.astype(FP8)
        lo = (a - hi.astype(np.float32)).astype(FP8)
        return hi, lo

    xT = np.ascontiguousarray(x.transpose(2, 0, 1))                 # [D,B,S]
    x_hi, x_lo = _fp8_pair(xT.reshape(NKT, 128, B, S))
    xT2 = np.stack([x_hi, x_lo], axis=1)                   # [kt,2,p,B,S]
    wqkvT = np.concatenate(
        [wq[h0 * HD:(h0 + HLOC) * HD] * scale,
         wk[kv * HD:(kv + 1) * HD],
         wv[kv * HD:(kv + 1) * HD]], axis=0).T                      # [D,384]
    w_hi, w_lo = _fp8_pair((wqkvT * 1024.0).reshape(NKT, 128, DQ + DKV))
    wqkv2 = np.stack([w_lo, w_hi], axis=2)                 # [kt,p,2,384]
    woT = np.ascontiguousarray(wo[:, h0 * HD:(h0 + HLOC) * HD].T)   # [DQ,D]

    n = np.arange(S, dtype=np.float32)
    kaug_ext = np.stack([n, np.ones(S, np.float32)])                # [2,S]

    qaug_ext = np.zeros((HLOC, 2, S), np.float32)
    for i in range(HLOC):
        sl = float(slopes[h0 + i])
        # stabilizer c[m] = max over allowed n of slope*(n-m), clipped >= 0
        c = np.maximum(0.0, np.maximum(sl * (cfg["n_hi"] - n),
                                       sl * (cfg["n_lo"] - n)))
        qaug_ext[i, 0, :] = sl
        qaug_ext[i, 1, :] = -sl * n - c

    ident = np.eye(64, dtype=np.float32)

    ins = {"xT2": xT2, "wqkv2": wqkv2, "woT": _bf16(woT),
           "kaug_ext": kaug_ext, "qaug_ext": qaug_ext,
           "ident": _bf16(ident)}
    if cfg["causal"]:
        ii = np.arange(128)[:, None]
        jj = np.arange(128)[None, :]
        ins["maskpat"] = _bf16(np.where(ii > jj, NEG, 0.0))
        ins["ident128"] = _bf16(np.eye(128))
    if cfg["generic_mask"]:
        ins["maskT"] = np.ascontiguousarray(mask[0, 0].T)
    return ins


def kernel(x, wq, wk, wv, wo, slopes, mask):
    from concourse.bass_utils import run_bass_kernel_spmd

    x = np.asarray(x, dtype=np.float32)
    wq = np.asarray(wq, dtype=np.float32)
    wk = np.asarray(wk, dtype=np.float32)
    wv = np.asarray(wv, dtype=np.float32)
    wo = np.asarray(wo, dtype=np.float32)
    slopes = np.asarray(slopes, dtype=np.float32)
    mask = np.asarray(mask, dtype=np.float32)

    B, S, D = x.shape
    HQ = 32
    HD = D // HQ
    n_cores = 8
    HLOC = HQ // n_cores

    causal, zeros, n_lo, n_hi = _analyze_mask(mask[0, 0], S)
    cfg = dict(B=B, S=S, D=D, HLOC=HLOC, HD=HD, MC=512,
               causal=causal, generic_mask=not (causal or zeros),
               n_lo=n_lo, n_hi=n_hi)

    nc = build_program(cfg)
    in_maps = [_make_inputs_for_core(c, x, wq, wk, wv, wo, slopes, mask, cfg)
               for c in range(n_cores)]
    res = run_bass_kernel_spmd(nc, in_maps, core_ids=list(range(n_cores)))
    out = np.zeros((B, S, D), np.float32)
    for c in range(n_cores):
        out += res.results[c]["out"].astype(np.float32)
    return out


if __name__ == "__main__":
    pass
